# revision 1
# baseline (speedup 1.0000x reference)
"""Bass/Trainium2 kernel for nn_BasicQuantumAttention (B=4, L=2048, d=512, 8 cores).

Sharding: core (b, s) = batch b, stream s (real/imag). Each core:
  - projects x[b] -> qT, kT (layout [d, L]) and v (layout [L, d]) in bf16,
    all SBUF-resident (no DRAM round-trip, no PE transposes: q/k are
    produced transposed by making W the stationary operand)
  - block-sparse masked attention for stream s with compile-time tile
    skipping at 128x128 (query x key) granularity over the union of the
    two masks (all 8 cores share one program); score->exp->attnV is
    software-pipelined one group ahead to keep PE fed
  - partial out-projection y^T_part = W_out^T[stream rows].T @ O_norm^T
Host sums the two partial y^T per batch and untransposes.

All matmuls are bf16 (1 cycle/row on PE, same as fp32r at free>=256, but
halves DMA/SBUF so everything stays resident); accumulation is fp32 PSUM.
"""
import sys

sys.path.insert(0, "/opt/trn_rl_repo")

import numpy as np
import ml_dtypes

import concourse.bass as bass
import concourse.tile as tile
from concourse import bacc, mybir
from concourse.bass_utils import run_bass_kernel_spmd

B, L, D = 4, 2048, 512
C6 = 6 * D            # 3072 input features
CT = C6 // 128        # 24 contraction tiles
QS = L // 512         # 4 query slices of 512 (normalization/out-proj grain)
QB = L // 128         # 16 query blocks of 128 (attention grain)
KT = L // 128         # 16 key tiles of 128
F32 = mybir.dt.float32
BF16 = mybir.dt.bfloat16
SCALE = float(D) ** -0.5
BF = ml_dtypes.bfloat16

# feature offsets inside qkv = [q_r q_i k_r k_i v_r v_i] (each D wide)
_Q_OFF = {0: 0 * D, 1: 1 * D}
_K_OFF = {0: 2 * D, 1: 3 * D}
_V_OFF = {0: 4 * D, 1: 5 * D}

LAST_RESULTS = None  # for test harness introspection


def build_program(kept, needs_mask, slot_of, n_slots, debug_phase="full",
                  zero_bias=False):
    """kept: {qb_global: [kt,...]} union keep lists at 128x128 granularity;
    needs_mask: set[(qb,kt)]; slot_of: {(qb,kt): slot}; n_slots >= 1."""
    nc = bacc.Bacc(None, target_bir_lowering=False, debug=False)

    x_t = nc.dram_tensor("x_t", [CT, 128, L], BF16, kind="ExternalInput")
    w_qk = nc.dram_tensor("w_qk", [CT, 128, 2, 512], BF16, kind="ExternalInput")
    w_v = nc.dram_tensor("w_v", [CT, 128, 512], BF16, kind="ExternalInput")
    w_o = nc.dram_tensor("w_o", [4, 128, 2 * D], BF16, kind="ExternalInput")
    b_qk = nc.dram_tensor("b_qk", [128, 8], F32, kind="ExternalInput")
    b_y = nc.dram_tensor("b_y", [128, 8], F32, kind="ExternalInput")
    mask_t = nc.dram_tensor("mask_t", [n_slots, 128, 128], BF16, kind="ExternalInput")
    ones_a = nc.dram_tensor("ones_a", [128, 1], BF16, kind="ExternalInput")
    ones_b = nc.dram_tensor("ones_b", [1, 128], BF16, kind="ExternalInput")
    b_yr = nc.dram_tensor("b_yr", [1, 2 * D], BF16, kind="ExternalInput")
    y = nc.dram_tensor("y", [2 * D, L], F32, kind="ExternalOutput")

    with tile.TileContext(nc) as tc, \
         nc.allow_low_precision(reason="bf16 matmuls within tolerance"):
        with tc.tile_pool(name="consts", bufs=1) as consts, \
             tc.tile_pool(name="kqv", bufs=1) as kqv:
            # const tiles allocated here; their DMAs are emitted inside the
            # projection block, after the first x piece on the gpsimd queue
            ones_k = consts.tile([128, 1], BF16)
            ones_1 = consts.tile([1, 128], BF16)
            bqk_s = consts.tile([128, 8], F32)
            by_s = consts.tile([128, 8], F32)
            byr_s = consts.tile([1, 2 * D], BF16)
            wo_sb = consts.tile([128, 4, 2 * D], BF16)

            qT_sb = kqv.tile([128, 4, L], BF16)
            kT_sb = kqv.tile([128, 4, L], BF16)
            v_sb = kqv.tile([128, KT, 512], BF16)

            # ---------------- projection phase ----------------
            with tc.tile_pool(name="wc", bufs=1) as wcp, \
                 tc.tile_pool(name="xin", bufs=2) as xp, \
                 tc.tile_pool(name="pp", bufs=8, space="PSUM") as pp:
                wqk_sb = wcp.tile([128, CT, 2, 512], BF16)
                wv_sb = wcp.tile([128, CT, 512], BF16)

                x_tiles = {}

                def load_x(qs_):
                    x_qs = xp.tile([128, CT, 512], BF16, name=f"x{qs_}", tag="x")
                    src = x_t[:, :, qs_ * 512:(qs_ + 1) * 512]
                    nc.sync.dma_start(
                        out=x_qs, in_=src.rearrange("ct p n -> p ct n"))
                    x_tiles[qs_] = x_qs

                # one hardware (SP) queue, in exact consumption order:
                # (x0[ct], wqk[ct]) pieces feed wave A progressively — small
                # pieces for the first cts (low first-matmul latency), then
                # 4-ct batches (HWDGE has ~0.3us fixed cost per transfer);
                # then wv for wave B, then the x[qs=1] slab. SWDGE queues
                # (gpsimd/scalar/vector) cost ~1us of engine time per DMA, so
                # bulk streams all ride the free HWDGE queue.
                # PE p-state warm-up: the tensor engine ramps 0.65->2.4GHz
                # over ~3us of elapsed time since its first instruction, so
                # burn the ramp on dummy matmuls while the first input DMAs
                # are still in flight (result is never read; the psum slot
                # is recycled by the first real chain)
                warm = consts.tile([128, 128], BF16)
                nc.vector.memset(warm, 0.0)
                wps = pp.tile([128, 512], F32, name="wps", tag="ps")
                for _ in range(8):
                    nc.tensor.matmul(wps[:, 0:128], warm[:, :], warm[:, :],
                                     start=True, stop=True)

                x0t = xp.tile([128, CT, 512], BF16, name="x0", tag="x")
                # first pieces ride three queues in parallel so the first
                # matmul unblocks in ~2us: x0[0] on gpsimd, wv[0] on scalar,
                # everything else streams on sync. Wave A is the v chains —
                # they need only x+wv (6.1 MB), which the serialized DMA
                # engines can deliver inside wave A's 20.5us of PE work;
                # q/k's 6.3 MB of wqk streams during the 41us of wave B.
                nc.gpsimd.dma_start(out=x0t[:, 0:1, :],
                                    in_=x_t[0:1, :, 0:512].rearrange(
                                        "ct p n -> p ct n"))
                # wv[0] heads the sync stream: the scalar (SWDGE) queue
                # needs ~2.5us just to issue one DMA, which would gate the
                # first matmul
                nc.sync.dma_start(
                    out=wv_sb[:, 0:1, :],
                    in_=w_v[0:1].rearrange("ct p d -> p ct d"))
                # consts trail the first x piece on gpsimd (nothing needs
                # them until the first evictions ~20us in)
                nc.gpsimd.dma_start(out=ones_k, in_=ones_a[:, :])
                nc.gpsimd.dma_start(out=ones_1, in_=ones_b[:, :])
                nc.gpsimd.dma_start(out=bqk_s, in_=b_qk[:, :])
                nc.gpsimd.dma_start(out=by_s, in_=b_y[:, :])
                nc.gpsimd.dma_start(out=byr_s, in_=b_yr[:, :])
                # pre-warm the exp activation table while PE does projection
                # (after the wv[0] issue: ACT processes its queue in order)
                scrap = consts.tile([128, 8], BF16)
                nc.scalar.activation(out=scrap, in_=bqk_s,
                                     func=mybir.ActivationFunctionType.Exp)
                # interleaved (x0, wv) stream in exact consumption order; the
                # transfer stream is serialized on the DMA engines, so
                # fewer/bigger pieces win over queue-splitting
                ct_groups = [list(range(c, min(c + 3, CT)))
                             for c in range(1, CT, 3)]
                for grp_ in ct_groups:
                    c0, cn = grp_[0], len(grp_)
                    nc.sync.dma_start(
                        out=x0t[:, c0:c0 + cn, :],
                        in_=x_t[c0:c0 + cn, :, 0:512].rearrange(
                            "ct p n -> p ct n"))
                    nc.sync.dma_start(
                        out=wv_sb[:, c0:c0 + cn, :],
                        in_=w_v[c0:c0 + cn].rearrange("ct p d -> p ct d"))
                for c0 in range(0, CT, 4):
                    nc.sync.dma_start(
                        out=wqk_sb[:, c0:c0 + 4, :, :],
                        in_=w_qk[c0:c0 + 4].rearrange("ct p f d -> p ct f d"))
                x_tiles[0] = x0t
                load_x(1)

                def evict(kind, ft, ps, qs_):
                    if kind == "v":
                        nc.scalar.copy(out=v_sb[:, qs_ * 4 + ft, :], in_=ps)
                    else:
                        dst = qT_sb if kind == "q" else kT_sb
                        bi = ft if kind == "q" else 4 + ft
                        if zero_bias:
                            nc.scalar.copy(
                                out=dst[:, ft, qs_ * 512:(qs_ + 1) * 512],
                                in_=ps)
                        else:
                            nc.scalar.activation(
                                out=dst[:, ft, qs_ * 512:(qs_ + 1) * 512],
                                in_=ps,
                                func=mybir.ActivationFunctionType.Identity,
                                bias=bqk_s[:, bi:bi + 1])

                def mm(kind, ft, ps, ct, x_qs, first, last):
                    if kind == "v":
                        nc.tensor.matmul(
                            ps[:, :], x_qs[:, ct, ft * 128:(ft + 1) * 128],
                            wv_sb[:, ct, :], start=first, stop=last)
                    else:
                        fc = 0 if kind == "q" else 1
                        nc.tensor.matmul(
                            ps[:, :], wqk_sb[:, ct, fc, ft * 128:(ft + 1) * 128],
                            x_qs[:, ct, :], start=first, stop=last)

                # qs=0: ct-major waves so PE consumption tracks DMA delivery
                x0 = x_tiles.pop(0)
                # wave A (3 chains, 0.64us/ct) stays under the serialized
                # DMA delivery rate (~0.73us/ct for x+wv) so PE never
                # starves during the input burst; the heavy q/k wave begins
                # right as the wqk stream lands
                waves = [[("v", nt) for nt in range(4)],
                         [("q", ft) for ft in range(4)] +
                         [("k", 0), ("k", 1)],
                         [("k", 2), ("k", 3)]]
                for wave in waves:
                    pss = {u: pp.tile([128, 512], F32, name=f"ps{u[0]}{u[1]}",
                                      tag="ps") for u in wave}
                    for ct in range(CT):
                        for u in wave:
                            mm(u[0], u[1], pss[u], ct, x0, ct == 0, ct == CT - 1)
                    for u in wave:
                        evict(u[0], u[1], pss[u], 0)

                for qs in range(1, QS):
                    if qs + 1 < QS:
                        load_x(qs + 1)
                    x_qs = x_tiles.pop(qs)
                    for kind in ("q", "k", "v"):
                        for ft in range(4):
                            ps = pp.tile([128, 512], F32, name="ps", tag="ps")
                            for ct in range(CT):
                                mm(kind, ft, ps, ct, x_qs, ct == 0, ct == CT - 1)
                            evict(kind, ft, ps, qs)

            # ---------------- attention + out-projection ----------------
            # per qs: flat list of (qb, kt) pairs chunked into units of 4
            if debug_phase == "dump":
                dbg_q = nc.dram_tensor("dbg_q", [4, 128, L], F32,
                                       kind="ExternalOutput")
                dbg_k = nc.dram_tensor("dbg_k", [4, 128, L], F32,
                                       kind="ExternalOutput")
                dbg_v = nc.dram_tensor("dbg_v", [16, 128, 512], F32,
                                       kind="ExternalOutput")
                with tc.tile_pool(name="dbg", bufs=2) as dbgp:
                    for ft in range(4):
                        for src, dst in ((qT_sb, dbg_q), (kT_sb, dbg_k)):
                            dt_ = dbgp.tile([128, L], F32, name="dt_", tag="d")
                            nc.scalar.copy(out=dt_, in_=src[:, ft, :])
                            nc.sync.dma_start(out=dst[ft], in_=dt_)
                    for nt in range(16):
                        dt_ = dbgp.tile([128, 512], F32, name="dt2", tag="d2")
                        nc.scalar.copy(out=dt_, in_=v_sb[:, nt, :])
                        nc.sync.dma_start(out=dbg_v[nt], in_=dt_)

            # attention work is organized per query-block (qb) "job": all of
            # the qb's score groups (4 kt per PSUM bank, sequential region
            # chains) are produced first, then attnV runs as 4+1 sequential
            # whole-klist chains. PSUM hardware allows only ONE open
            # accumulation group per 2KB bank — interleaved per-region
            # starts corrupt earlier regions' accumulation.
            jobs = []   # (qs, qb, [groups of up to 4 kt])
            for qs in range(QS):
                for qb in range(4):
                    klist = kept[qs * 4 + qb]
                    jobs.append((qs, qb,
                                 [klist[i:i + 4]
                                  for i in range(0, len(klist), 4)]))

            dbg_ot = dbg_dp = None
            if debug_phase == "dumpot":
                dbg_ot = nc.dram_tensor("dbg_ot", [16, 128, 4, 128], F32,
                                        kind="ExternalOutput")
                dbg_dp = nc.dram_tensor("dbg_dp", [16, 1, 128], F32,
                                        kind="ExternalOutput")
            with tc.tile_pool(name="sy", bufs=3, space="PSUM") as syp, \
                 tc.tile_pool(name="op", bufs=2, space="PSUM") as opp, \
                 tc.tile_pool(name="dn", bufs=1, space="PSUM") as dnp, \
                 tc.tile_pool(name="yp", bufs=2, space="PSUM") as ypp, \
                 tc.tile_pool(name="pt", bufs=8) as ptp, \
                 tc.tile_pool(name="mk", bufs=8) as mkp, \
                 tc.tile_pool(name="ot", bufs=3) as otp, \
                 tc.tile_pool(name="sm", bufs=2) as smp, \
                 tc.tile_pool(name="yo", bufs=4) as yop:
                nc.scalar.dma_start(
                    out=wo_sb, in_=w_o.rearrange("ft p g -> p ft g"))

                state = {}   # per-qs tiles: ot, dps; per-(qs,qb): ops

                def emit_scores_grp(qs, qb, grp):
                    g = qs * 4 + qb
                    w = len(grp)
                    sps = syp.tile([128, 4, 128], F32, name="sps", tag="sps")
                    for j, kt in enumerate(grp):
                        for dt in range(4):
                            nc.tensor.matmul(
                                sps[:, j, :],
                                kT_sb[:, dt, kt * 128:(kt + 1) * 128],
                                qT_sb[:, dt, g * 128:(g + 1) * 128],
                                start=(dt == 0), stop=(dt == 3))
                    pT = ptp.tile([128, 4, 128], BF16, name="pT", tag="pT")
                    nc.scalar.activation(
                        out=pT[:, :w, :], in_=sps[:, :w, :],
                        func=mybir.ActivationFunctionType.Exp, scale=SCALE)
                    masked = [j for j, kt in enumerate(grp)
                              if (g, kt) in needs_mask]
                    if masked:
                        mt = mkp.tile([128, 4, 128], BF16, name="mt", tag="mt")
                        slots = [slot_of[(g, grp[j])] for j in masked]
                        contig = (len(masked) == masked[-1] - masked[0] + 1
                                  and slots == list(range(slots[0],
                                                          slots[0] + len(slots))))
                        if contig:
                            # one DMA per group: queue sequencers cost ~1.26us
                            # per dma_start, so per-slot DMAs would starve
                            j0, sw = masked[0], len(masked)
                            nc.sync.dma_start(
                                out=mt[:, j0:j0 + sw, :],
                                in_=mask_t[slots[0]:slots[0] + sw].rearrange(
                                    "s p n -> p s n"))
                        else:
                            for i, j in enumerate(masked):
                                nc.sync.dma_start(out=mt[:, j, :],
                                                  in_=mask_t[slots[i]])
                        if len(masked) == w:
                            nc.vector.tensor_mul(
                                pT[:, :w, :], pT[:, :w, :], mt[:, :w, :])
                        else:
                            for j in masked:
                                nc.vector.tensor_mul(
                                    pT[:, j, :], pT[:, j, :], mt[:, j, :])
                    return pT

                def emit_final_recip(qs, qb):
                    """DVE part of the final-qs per-qb head; emitted right
                    after the dps chain so it overlaps the ops chains."""
                    dps = state[("dps", qs)]
                    c0, c1 = qb * 128, (qb + 1) * 128
                    recf = smp.tile([1, 128], F32, tag="recf", name="recf")
                    nc.vector.reciprocal(recf, dps[:, c0:c1])
                    recs = smp.tile([1, 128], BF16, tag="recs", name="recs")
                    nc.vector.tensor_copy(out=recs, in_=recf)
                    return recs

                def emit_final_qb_head(qs, qb, recs):
                    """Per-qb head + out-projection for the final qs: emitted
                    inline as each qb finishes, so only qb=3's slice remains
                    on the critical path at program end. Normalization goes
                    into ot (bf16, cheap on DVE); the y bias rides the PE as
                    a rank-1 matmul so evictions batch into single copies."""
                    ot = state[("ot", qs)]
                    c0, c1 = qb * 128, (qb + 1) * 128
                    bps = syp.tile([128, 128], F32, tag="sps", name="bps")
                    nc.tensor.matmul(bps[:, :], ones_1[:, :], recs[:, :],
                                     start=True, stop=True)
                    rb = smp.tile([128, 128], BF16, tag="rb", name="rb")
                    nc.scalar.copy(out=rb, in_=bps)
                    for ft in range(4):
                        nc.vector.tensor_mul(ot[:, ft, c0:c1],
                                             ot[:, ft, c0:c1], rb)
                    for gh in range(2):
                        ypsg = ypp.tile([128, 4, 128], F32, tag="yp", name="ypsg")
                        for gi in range(4):
                            gt = gh * 4 + gi
                            for ft in range(4):
                                nc.tensor.matmul(
                                    ypsg[:, gi, :],
                                    wo_sb[:, ft, gt * 128:(gt + 1) * 128],
                                    ot[:, ft, c0:c1],
                                    start=(ft == 0),
                                    stop=(zero_bias and ft == 3))
                            if not zero_bias:
                                nc.tensor.matmul(
                                    ypsg[:, gi, :],
                                    byr_s[:, gt * 128:(gt + 1) * 128],
                                    ones_1[:, :], start=False, stop=True)
                        y_sbq = yop.tile([128, 4, 128], F32,
                                         name="ysq", tag="ysq")
                        if qb == 3 and gh == 1:
                            # parallel final evictions: ACT does gh0, DVE gh1
                            nc.vector.tensor_copy(out=y_sbq, in_=ypsg)
                        else:
                            nc.scalar.copy(out=y_sbq, in_=ypsg)
                        yq = (nc.sync, nc.scalar)[gh] if qb == 3 else \
                            (nc.sync, nc.gpsimd)[gh]
                        yq.dma_start(
                            out=y[gh * 512:(gh + 1) * 512,
                                  qs * 512 + c0:qs * 512 + c1].rearrange(
                                      "(a p) n -> p a n", p=128),
                            in_=y_sbq)

                def emit_attnv_job(job, pts):
                    """4+1 sequential whole-klist accumulation chains for one
                    query block; each chain is a single open PSUM group. The
                    dps (row-sum) chain goes first so the reciprocal path of
                    the final-qs head overlaps the ops chains."""
                    qs, qb, groups = job
                    dps = state[("dps", qs)]
                    n = sum(len(g) for g in groups)
                    i = 0
                    for gi, grp in enumerate(groups):
                        for j, kt in enumerate(grp):
                            nc.tensor.matmul(
                                dps[:, qb * 128:(qb + 1) * 128],
                                ones_k[:, :], pts[gi][:, j, :],
                                start=(i == 0), stop=(i == n - 1))
                            i += 1
                    recs = None
                    if qs == QS - 1:
                        recs = emit_final_recip(qs, qb)
                    ops = opp.tile([128, 4, 128], F32, name="ops", tag="ops")
                    for dvt in range(4):
                        i = 0
                        for gi, grp in enumerate(groups):
                            for j, kt in enumerate(grp):
                                nc.tensor.matmul(
                                    ops[:, dvt, :],
                                    v_sb[:, kt, dvt * 128:(dvt + 1) * 128],
                                    pts[gi][:, j, :],
                                    start=(i == 0), stop=(i == n - 1))
                                i += 1
                    nc.vector.tensor_copy(
                        out=state[("ot", qs)][:, :, qb * 128:(qb + 1) * 128],
                        in_=ops)
                    if dbg_ot is not None:
                        g2 = qs * 4 + qb
                        dt_ = yop.tile([128, 4, 128], F32, name="dto", tag="dto")
                        nc.scalar.copy(
                            out=dt_,
                            in_=state[("ot", qs)][:, :, qb * 128:(qb + 1) * 128])
                        nc.gpsimd.dma_start(out=dbg_ot[g2], in_=dt_)
                        dd = smp.tile([1, 128], F32, name="ddp", tag="ddp")
                        nc.vector.tensor_copy(
                            out=dd, in_=dps[:, qb * 128:(qb + 1) * 128])
                        nc.gpsimd.dma_start(out=dbg_dp[g2], in_=dd)
                    if qs == QS - 1:
                        emit_final_qb_head(qs, qb, recs)

                # head: normalization happens AFTER the out-projection
                # (per-query scaling commutes with the d-mixing matmul), so
                # the yps chains never wait on the reciprocal chain. Stage a
                # (DVE recip) fires at the last attnV; stage b (everything
                # else) one unit later so the bps broadcast never
                # head-of-line-blocks the PE queue.
                def head_a(qs):
                    dps = state.pop(("dps", qs))
                    recip = smp.tile([1, 512], F32, tag="recf", name="recip")
                    nc.vector.reciprocal(recip, dps)
                    recs = smp.tile([1, 512], BF16, tag="recs", name="recs")
                    nc.vector.tensor_copy(out=recs, in_=recip)
                    state[("recs", qs)] = recs

                def head_b(qs):
                    ot = state.pop(("ot", qs))
                    recs = state.pop(("recs", qs))
                    bps = ypp.tile([128, 512], F32, tag="yp", name="bps")
                    nc.tensor.matmul(bps[:, :], ones_1[:, :], recs[:, :],
                                     start=True, stop=True)
                    rb = smp.tile([128, 512], F32, tag="rb", name="rb")
                    nc.scalar.copy(out=rb, in_=bps)
                    # y DMAs alternate queues (each queue sequencer costs
                    # ~1.26us per dma_start): keep sync mask-free except for
                    # the final qs, whose masks are all done
                    yqs = ([nc.sync, nc.gpsimd, nc.scalar, nc.sync]
                           if qs == QS - 1 else
                           [nc.gpsimd, nc.scalar, nc.gpsimd, nc.scalar])
                    for gp in range(4):
                        y_sb = yop.tile([128, 2, 512], F32,
                                        name="y_sb", tag="y_sb")
                        for gi in range(2):
                            gt = gp * 2 + gi
                            yps = ypp.tile([128, 512], F32, tag="yp", name="yps")
                            for ft in range(4):
                                nc.tensor.matmul(
                                    yps[:, :],
                                    wo_sb[:, ft, gt * 128:(gt + 1) * 128],
                                    ot[:, ft, :], start=(ft == 0), stop=(ft == 3))
                            ym = yop.tile([128, 512], F32, name="ym", tag="ym")
                            nc.vector.tensor_mul(ym, yps, rb)
                            if zero_bias:
                                nc.scalar.copy(out=y_sb[:, gi, :], in_=ym)
                            else:
                                nc.scalar.activation(
                                    out=y_sb[:, gi, :], in_=ym,
                                    func=mybir.ActivationFunctionType.Identity,
                                    bias=by_s[:, gt:gt + 1])
                        yqs[gp].dma_start(
                            out=y[gp * 256:(gp + 1) * 256,
                                  qs * 512:(qs + 1) * 512].rearrange(
                                      "(a p) n -> p a n", p=128),
                            in_=y_sb)

                # software pipeline at qb-job granularity: scores for job
                # i+1 are emitted before the attnV chains of job i, so PE
                # always has queued score work covering exp/mask and head
                # latencies
                heads = []
                HEAD_STAGES = (head_a, head_b)

                def step_heads():
                    for h in list(heads):
                        HEAD_STAGES[h[1]](h[0])
                        h[1] += 1
                        if h[1] == len(HEAD_STAGES):
                            heads.remove(h)

                def retire(job, pts):
                    emit_attnv_job(job, pts)
                    qs, qb = job[0], job[1]
                    if qb == 3:
                        if qs == QS - 1:
                            state.pop(("ot", qs), None)
                            state.pop(("dps", qs), None)
                        else:
                            heads.append([qs, 0])
                    step_heads()

                prev = None
                for job in jobs:
                    qs, qb, groups = job
                    if ("ot", qs) not in state:
                        state[("ot", qs)] = otp.tile(
                            [128, 4, 512], BF16, name="ot", tag="ot")
                        state[("dps", qs)] = dnp.tile(
                            [1, 512], F32, name="dps", tag="dps")
                    pts = [emit_scores_grp(qs, qb, grp) for grp in groups]
                    if prev is not None:
                        retire(*prev)
                    prev = (job, pts)
                retire(*prev)
                while heads:
                    step_heads()

    nc.compile()
    return nc


def _prep_masks(mask_real, mask_imag):
    """Compile-time 128x128 tile analysis + per-core mask slot data."""
    mts = [np.ascontiguousarray(np.asarray(m).T) for m in (mask_real, mask_imag)]
    kept = {}
    needs_mask = set()
    slot_of = {}
    slots = []  # (qb, kt)
    for g in range(QB):
        klist = []
        for kt in range(KT):
            subs = [m[kt * 128:(kt + 1) * 128, g * 128:(g + 1) * 128] for m in mts]
            anys = [s.any() for s in subs]
            alls = [s.all() for s in subs]
            if not (anys[0] or anys[1]):
                continue
            klist.append(kt)
            if not (alls[0] and alls[1]):
                needs_mask.add((g, kt))
                slot_of[(g, kt)] = len(slots)
                slots.append((g, kt))
        kept[g] = klist
    n_slots = max(1, len(slots))
    mask_data = []
    for s in range(2):
        md = np.zeros((n_slots, 128, 128), BF)
        for i, (g, kt) in enumerate(slots):
            md[i] = mts[s][kt * 128:(kt + 1) * 128,
                           g * 128:(g + 1) * 128].astype(BF)
        mask_data.append(md)
    return kept, needs_mask, slot_of, n_slots, mask_data


def kernel(q_real, q_imag, k_real, k_imag, v_real, v_imag,
           W_qkv, b_qkv, W_out, b_out, mask_real, mask_imag, _trace=False):
    global LAST_RESULTS
    args = [np.asarray(a) for a in (q_real, q_imag, k_real, k_imag, v_real, v_imag)]
    W_qkv = np.asarray(W_qkv, np.float32)
    b_qkv = np.asarray(b_qkv, np.float32)
    W_out = np.asarray(W_out, np.float32)
    b_out = np.asarray(b_out, np.float32)

    kept, needs_mask, slot_of, n_slots, mask_data = _prep_masks(mask_real, mask_imag)
    zb = bool(not b_qkv.any() and not b_out.any())
    nc = build_program(kept, needs_mask, slot_of, n_slots, zero_bias=zb)

    # x^T per batch, c-tiled: [CT, 128, L] in bf16
    x_ts = []
    for b in range(B):
        xb = np.concatenate([a[b] for a in args], axis=1)          # [L, 6D]
        xt = np.ascontiguousarray(xb.T.astype(BF))                  # [6D, L]
        x_ts.append(np.ascontiguousarray(xt.reshape(CT, 128, L)))

    W6T = W_qkv.T  # [c, f]
    w_qks, w_vs, b_qks, w_os, b_ys = [], [], [], [], []
    W2T = W_out.T  # [f=2D, g=2D]
    for s in range(2):
        wq = W6T[:, _Q_OFF[s]:_Q_OFF[s] + D].reshape(CT, 128, 512)
        wk = W6T[:, _K_OFF[s]:_K_OFF[s] + D].reshape(CT, 128, 512)
        wv = W6T[:, _V_OFF[s]:_V_OFF[s] + D].reshape(CT, 128, 512)
        w_qks.append(np.ascontiguousarray(
            np.stack([wq, wk], axis=2).astype(BF)))                 # [CT,128,2,512]
        w_vs.append(np.ascontiguousarray(wv.astype(BF)))            # [CT,128,512]
        bq = b_qkv[_Q_OFF[s]:_Q_OFF[s] + D].reshape(4, 128).T
        bk = b_qkv[_K_OFF[s]:_K_OFF[s] + D].reshape(4, 128).T
        b_qks.append(np.ascontiguousarray(
            np.concatenate([bq, bk], axis=1), dtype=np.float32))    # [128, 8]

        w_os.append(np.ascontiguousarray(
            W2T[s * D:(s + 1) * D, :].reshape(4, 128, 2 * D).astype(BF)))
        if s == 0:
            b_v_cat = np.concatenate([b_qkv[_V_OFF[0]:_V_OFF[0] + D],
                                      b_qkv[_V_OFF[1]:_V_OFF[1] + D]])
            b_eff = (W_out @ b_v_cat + b_out).astype(np.float32)
            b_ys.append(np.ascontiguousarray(b_eff.reshape(8, 128).T))
        else:
            b_ys.append(np.zeros((128, 8), np.float32))

    in_maps = []
    for core in range(8):
        b, s = core // 2, core % 2
        in_maps.append({
            "x_t": x_ts[b], "w_qk": w_qks[s], "w_v": w_vs[s], "w_o": w_os[s],
            "b_qk": b_qks[s], "b_y": b_ys[s],
            "b_yr": np.ascontiguousarray(
                b_ys[s].T.reshape(1, 2 * D).astype(BF)),
            "mask_t": mask_data[s],
            "ones_a": np.ones((128, 1), BF),
            "ones_b": np.ones((1, 128), BF),
        })

    res = run_bass_kernel_spmd(nc, in_maps, core_ids=list(range(8)), trace=_trace)
    LAST_RESULTS = res

    out_real = np.empty((B, L, D), np.float32)
    out_imag = np.empty((B, L, D), np.float32)
    for b in range(B):
        yt = res.results[2 * b]["y"] + res.results[2 * b + 1]["y"]  # [2D, L]
        yb = yt.T                                                   # [L, 2D]
        out_real[b] = yb[:, :D]
        out_imag[b] = yb[:, D:]
    return out_real, out_imag



# revision 4
# speedup vs baseline: 1.1881x; 1.1881x over previous
"""Bass/Trainium2 kernel for nn_BasicQuantumAttention (B=4, L=2048, d=512, 8 cores).

Sharding: core (b, s) = batch b, stream s (real/imag); one program per
stream (each stream's own block-sparse keep-set; ~52/60 kept 128x128
tiles vs 79 for the union). Each core:
  - projects x[b] -> qT, kT (layout [d, L]) and v (layout [L, d]), all
    SBUF-resident. The projection runs as fp8 DoubleRow matmuls with
    hi/lo error compensation: every operand A is split into
    A_hi = e4m3(A) and A_lo = e5m2(A - A_hi), and A@B is computed as
    three DoubleRow pass chains (Ah@Bh, Ah@Bl, Al@Bh). DoubleRow
    contracts two 128-K slabs per instruction, so the three passes cost
    0.75x the bf16 cycles while matching bf16 accuracy (the dropped
    Al@Bl term is ~2^-8 relative). q/k are evicted from PSUM as
    (e4m3 hi, e5m2 lo) pairs so the score matmuls use the same scheme;
    v is evicted bf16 for the (bf16) attnV matmuls.
  - block-sparse masked attention with compile-time tile skipping at
    128x128 granularity on this stream's mask; scores are fp8 tri-term
    DoubleRow, exp/mask/attnV as in the bf16 kernel.
  - partial out-projection y^T_part = W_out^T[stream rows].T @ O_norm^T
    (bf16).
Host sums the two partial y^T per batch and untransposes.
"""
import sys

sys.path.insert(0, "/opt/trn_rl_repo")

import numpy as np
import ml_dtypes

import concourse.bass as bass
import concourse.tile as tile
from concourse import bacc, mybir
from concourse.bass_utils import run_bass_kernel_spmd

B, L, D = 4, 2048, 512
C6 = 6 * D            # 3072 input features
CT = C6 // 128        # 24 contraction tiles
QS = L // 512         # 4 query slices of 512 (normalization/out-proj grain)
QB = L // 128         # 16 query blocks of 128 (attention grain)
KT = L // 128         # 16 key tiles of 128
F32 = mybir.dt.float32
BF16 = mybir.dt.bfloat16
F8H = mybir.dt.float8e4
F8L = mybir.dt.float8e5
DR = mybir.MatmulPerfMode.DoubleRow
SCALE = float(D) ** -0.5
BF = ml_dtypes.bfloat16
H8 = ml_dtypes.float8_e4m3
E5 = ml_dtypes.float8_e5m2

# feature offsets inside qkv = [q_r q_i k_r k_i v_r v_i] (each D wide)
_Q_OFF = {0: 0 * D, 1: 1 * D}
_K_OFF = {0: 2 * D, 1: 3 * D}
_V_OFF = {0: 4 * D, 1: 5 * D}

LAST_RESULTS = None   # list of per-stream BassKernelResults
LAST_PROGRAMS = None  # list of per-stream compiled Bacc programs


def build_program(kept, needs_mask, slot_of, n_slots, zero_bias=False):
    """kept: {qb_global: [kt,...]} keep lists at 128x128 granularity for
    THIS stream; needs_mask: set[(qb,kt)]; slot_of: {(qb,kt): slot}."""
    nc = bacc.Bacc(None, target_bir_lowering=False, debug=False)

    x8_t = nc.dram_tensor("x8_t", [CT, 128, L], F8H, kind="ExternalInput")
    xl_t = nc.dram_tensor("xl_t", [CT, 128, L], F8L, kind="ExternalInput")
    wqkh_t = nc.dram_tensor("wqkh_t", [CT, 128, 2, 512], F8H, kind="ExternalInput")
    wqkl_t = nc.dram_tensor("wqkl_t", [CT, 128, 2, 512], F8L, kind="ExternalInput")
    wvh_t = nc.dram_tensor("wvh_t", [CT, 128, 512], F8H, kind="ExternalInput")
    wvl_t = nc.dram_tensor("wvl_t", [CT, 128, 512], F8L, kind="ExternalInput")
    w_o = nc.dram_tensor("w_o", [4, 128, 2 * D], BF16, kind="ExternalInput")
    b_qk = nc.dram_tensor("b_qk", [128, 8], F32, kind="ExternalInput")
    b_y = nc.dram_tensor("b_y", [128, 8], F32, kind="ExternalInput")
    mask_t = nc.dram_tensor("mask_t", [n_slots, 128, 128], BF16, kind="ExternalInput")
    ones_a = nc.dram_tensor("ones_a", [128, 1], BF16, kind="ExternalInput")
    ones_b = nc.dram_tensor("ones_b", [1, 128], BF16, kind="ExternalInput")
    b_yr = nc.dram_tensor("b_yr", [1, 2 * D], BF16, kind="ExternalInput")
    y = nc.dram_tensor("y", [2 * D, L], F32, kind="ExternalOutput")

    with tile.TileContext(nc) as tc, \
         nc.allow_low_precision(reason="fp8 hi/lo compensated matmuls"):
        with tc.tile_pool(name="consts", bufs=1) as consts, \
             tc.tile_pool(name="kqv", bufs=1) as kqv:
            ones_k = consts.tile([128, 1], BF16)
            ones_1 = consts.tile([1, 128], BF16)
            bqk_s = consts.tile([128, 8], F32)
            by_s = consts.tile([128, 8], F32)
            byr_s = consts.tile([1, 2 * D], BF16)
            wo_sb = consts.tile([128, 4, 2 * D], BF16)

            qh_sb = kqv.tile([128, 4, L], F8H)
            ql_sb = kqv.tile([128, 4, L], F8L)
            kh_sb = kqv.tile([128, 4, L], F8H)
            kl_sb = kqv.tile([128, 4, L], F8L)
            v_sb = kqv.tile([128, KT, 512], BF16)

            # ---------------- projection phase ----------------
            with tc.tile_pool(name="wc", bufs=1) as wcp, \
                 tc.tile_pool(name="xin", bufs=2) as xp, \
                 tc.tile_pool(name="ev", bufs=4) as evp, \
                 tc.tile_pool(name="pp", bufs=8, space="PSUM") as pp:
                wqkh_sb = wcp.tile([128, CT, 2, 512], F8H)
                wqkl_sb = wcp.tile([128, CT, 2, 512], F8L)
                wvh_sb = wcp.tile([128, CT, 512], F8H)
                wvl_sb = wcp.tile([128, CT, 512], F8L)

                x_tiles = {}

                def load_x(qs_):
                    xh = xp.tile([128, CT, 512], F8H, name=f"x8{qs_}", tag="x8")
                    xl = xp.tile([128, CT, 512], F8L, name=f"xl{qs_}", tag="xl")
                    sl = slice(qs_ * 512, (qs_ + 1) * 512)
                    nc.sync.dma_start(
                        out=xh, in_=x8_t[:, :, sl].rearrange("ct p n -> p ct n"))
                    nc.sync.dma_start(
                        out=xl, in_=xl_t[:, :, sl].rearrange("ct p n -> p ct n"))
                    x_tiles[qs_] = (xh, xl)

                # PE p-state warm-up: burn the 0.65->2.4GHz ramp on dummy
                # matmuls while the first input DMAs are in flight
                warm = consts.tile([128, 128], BF16)
                nc.vector.memset(warm, 0.0)
                wps = pp.tile([128, 512], F32, name="wps", tag="ps")
                for _ in range(8):
                    nc.tensor.matmul(wps[:, 0:128], warm[:, :], warm[:, :],
                                     start=True, stop=True)

                x0h = xp.tile([128, CT, 512], F8H, name="x80", tag="x8")
                x0l = xp.tile([128, CT, 512], F8L, name="xl0", tag="xl")
                # first pieces ride two queues in parallel so the first
                # DoubleRow matmul (needs x8[0:2] + wvh[0:2]) unblocks fast:
                # x8[0:2] on gpsimd, wvh/wvl/xl[0:2] at the sync head, then
                # interleaved 3-ct groups in exact wave-A consumption order,
                # then the wqk hi/lo streams for waves B/C, then x[qs=1].
                nc.gpsimd.dma_start(out=x0h[:, 0:2, :],
                                    in_=x8_t[0:2, :, 0:512].rearrange(
                                        "ct p n -> p ct n"))
                nc.sync.dma_start(
                    out=wvh_sb[:, 0:2, :],
                    in_=wvh_t[0:2].rearrange("ct p d -> p ct d"))
                nc.sync.dma_start(
                    out=wvl_sb[:, 0:2, :],
                    in_=wvl_t[0:2].rearrange("ct p d -> p ct d"))
                nc.sync.dma_start(out=x0l[:, 0:2, :],
                                  in_=xl_t[0:2, :, 0:512].rearrange(
                                      "ct p n -> p ct n"))
                # consts trail on gpsimd (nothing needs them until the first
                # evictions ~15us in)
                nc.gpsimd.dma_start(out=ones_k, in_=ones_a[:, :])
                nc.gpsimd.dma_start(out=ones_1, in_=ones_b[:, :])
                nc.gpsimd.dma_start(out=bqk_s, in_=b_qk[:, :])
                nc.gpsimd.dma_start(out=by_s, in_=b_y[:, :])
                nc.gpsimd.dma_start(out=byr_s, in_=b_yr[:, :])
                # pre-warm the exp activation table while PE projects
                scrap = consts.tile([128, 8], BF16)
                nc.scalar.activation(out=scrap, in_=bqk_s,
                                     func=mybir.ActivationFunctionType.Exp)
                ct_groups = [list(range(c, min(c + 3, CT)))
                             for c in range(2, CT, 3)]
                for grp_ in ct_groups:
                    c0, cn = grp_[0], len(grp_)
                    nc.sync.dma_start(
                        out=x0h[:, c0:c0 + cn, :],
                        in_=x8_t[c0:c0 + cn, :, 0:512].rearrange(
                            "ct p n -> p ct n"))
                    nc.sync.dma_start(
                        out=x0l[:, c0:c0 + cn, :],
                        in_=xl_t[c0:c0 + cn, :, 0:512].rearrange(
                            "ct p n -> p ct n"))
                    nc.sync.dma_start(
                        out=wvh_sb[:, c0:c0 + cn, :],
                        in_=wvh_t[c0:c0 + cn].rearrange("ct p d -> p ct d"))
                    nc.sync.dma_start(
                        out=wvl_sb[:, c0:c0 + cn, :],
                        in_=wvl_t[c0:c0 + cn].rearrange("ct p d -> p ct d"))
                for c0 in range(0, CT, 4):
                    nc.sync.dma_start(
                        out=wqkh_sb[:, c0:c0 + 4, :, :],
                        in_=wqkh_t[c0:c0 + 4].rearrange("ct p f d -> p ct f d"))
                for c0 in range(0, CT, 4):
                    nc.sync.dma_start(
                        out=wqkl_sb[:, c0:c0 + 4, :, :],
                        in_=wqkl_t[c0:c0 + 4].rearrange("ct p f d -> p ct f d"))
                x_tiles[0] = (x0h, x0l)
                load_x(1)

                def evict(kind, ft, ps, qs_):
                    if kind == "v":
                        # v bias is folded into b_y on the host (as in the
                        # bf16 kernel), so v eviction is always a plain copy
                        nc.scalar.copy(out=v_sb[:, qs_ * 4 + ft, :], in_=ps)
                        return
                    hi, lo = (qh_sb, ql_sb) if kind == "q" else (kh_sb, kl_sb)
                    bi = ft if kind == "q" else 4 + ft
                    sl = slice(qs_ * 512, (qs_ + 1) * 512)
                    if zero_bias:
                        nc.scalar.copy(out=hi[:, ft, sl], in_=ps)
                        nc.vector.tensor_sub(lo[:, ft, sl], ps, hi[:, ft, sl])
                    else:
                        tmp = evp.tile([128, 512], BF16, name="evt", tag="evt")
                        nc.scalar.activation(
                            out=tmp, in_=ps,
                            func=mybir.ActivationFunctionType.Identity,
                            bias=bqk_s[:, bi:bi + 1])
                        nc.scalar.copy(out=hi[:, ft, sl], in_=tmp)
                        nc.vector.tensor_sub(lo[:, ft, sl], tmp, hi[:, ft, sl])

                def mm_steps(kind, ft, c, xt):
                    """The 3 DoubleRow (lhsT, rhs) pairs for ct-pair c."""
                    xh, xl = xt
                    cp = slice(c, c + 2)
                    fsl = slice(ft * 128, (ft + 1) * 128)
                    if kind == "v":
                        return [(xh[:, cp, fsl], wvh_sb[:, cp, :]),
                                (xh[:, cp, fsl], wvl_sb[:, cp, :]),
                                (xl[:, cp, fsl], wvh_sb[:, cp, :])]
                    fc = 0 if kind == "q" else 1
                    return [(wqkh_sb[:, cp, fc, fsl], xh[:, cp, :]),
                            (wqkl_sb[:, cp, fc, fsl], xh[:, cp, :]),
                            (wqkh_sb[:, cp, fc, fsl], xl[:, cp, :])]

                NP = 3 * (CT // 2)  # matmuls per chain

                # qs=0: ct-pair-major waves so PE consumption tracks DMA
                # delivery; wave A (v) needs only x+wv, waves B/C need wqk
                x0 = x_tiles.pop(0)
                waves = [[("v", nt) for nt in range(4)],
                         [("q", ft) for ft in range(4)] +
                         [("k", 0), ("k", 1)],
                         [("k", 2), ("k", 3)]]
                for wave in waves:
                    pss = {u: pp.tile([128, 512], F32, name=f"ps{u[0]}{u[1]}",
                                      tag="ps") for u in wave}
                    cnt = {u: 0 for u in wave}
                    for c in range(0, CT, 2):
                        for u in wave:
                            for lhsT, rhs in mm_steps(u[0], u[1], c, x0):
                                nc.tensor.matmul(
                                    pss[u][:, :], lhsT, rhs,
                                    start=(cnt[u] == 0),
                                    stop=(cnt[u] == NP - 1), perf_mode=DR)
                                cnt[u] += 1
                    for u in wave:
                        evict(u[0], u[1], pss[u], 0)

                for qs in range(1, QS):
                    if qs + 1 < QS:
                        load_x(qs + 1)
                    x_qs = x_tiles.pop(qs)
                    for kind in ("q", "k", "v"):
                        for ft in range(4):
                            ps = pp.tile([128, 512], F32, name="ps", tag="ps")
                            i = 0
                            for c in range(0, CT, 2):
                                for lhsT, rhs in mm_steps(kind, ft, c, x_qs):
                                    nc.tensor.matmul(
                                        ps[:, :], lhsT, rhs,
                                        start=(i == 0), stop=(i == NP - 1),
                                        perf_mode=DR)
                                    i += 1
                            evict(kind, ft, ps, qs)

            # ---------------- attention + out-projection ----------------
            jobs = []   # (qs, qb, [groups of up to 4 kt])
            for qs in range(QS):
                for qb in range(4):
                    klist = kept[qs * 4 + qb]
                    jobs.append((qs, qb,
                                 [klist[i:i + 4]
                                  for i in range(0, len(klist), 4)]))

            with tc.tile_pool(name="sy", bufs=3, space="PSUM") as syp, \
                 tc.tile_pool(name="op", bufs=2, space="PSUM") as opp, \
                 tc.tile_pool(name="dn", bufs=1, space="PSUM") as dnp, \
                 tc.tile_pool(name="yp", bufs=2, space="PSUM") as ypp, \
                 tc.tile_pool(name="pt", bufs=8) as ptp, \
                 tc.tile_pool(name="mk", bufs=8) as mkp, \
                 tc.tile_pool(name="ot", bufs=3) as otp, \
                 tc.tile_pool(name="sm", bufs=2) as smp, \
                 tc.tile_pool(name="yo", bufs=4) as yop:
                nc.scalar.dma_start(
                    out=wo_sb, in_=w_o.rearrange("ft p g -> p ft g"))

                state = {}   # per-qs tiles: ot, dps

                def emit_scores_grp(qs, qb, grp):
                    g = qs * 4 + qb
                    w = len(grp)
                    gsl = slice(g * 128, (g + 1) * 128)
                    sps = syp.tile([128, 4, 128], F32, name="sps", tag="sps")
                    for j, kt in enumerate(grp):
                        ksl = slice(kt * 128, (kt + 1) * 128)
                        steps = []
                        for dt in (0, 2):
                            steps.append((kh_sb[:, dt:dt + 2, ksl],
                                          qh_sb[:, dt:dt + 2, gsl]))
                        for dt in (0, 2):
                            steps.append((kh_sb[:, dt:dt + 2, ksl],
                                          ql_sb[:, dt:dt + 2, gsl]))
                        for dt in (0, 2):
                            steps.append((kl_sb[:, dt:dt + 2, ksl],
                                          qh_sb[:, dt:dt + 2, gsl]))
                        for i, (lhsT, rhs) in enumerate(steps):
                            nc.tensor.matmul(
                                sps[:, j, :], lhsT, rhs,
                                start=(i == 0), stop=(i == len(steps) - 1),
                                perf_mode=DR)
                    pT = ptp.tile([128, 4, 128], BF16, name="pT", tag="pT")
                    nc.scalar.activation(
                        out=pT[:, :w, :], in_=sps[:, :w, :],
                        func=mybir.ActivationFunctionType.Exp, scale=SCALE)
                    masked = [j for j, kt in enumerate(grp)
                              if (g, kt) in needs_mask]
                    if masked:
                        mt = mkp.tile([128, 4, 128], BF16, name="mt", tag="mt")
                        slots = [slot_of[(g, grp[j])] for j in masked]
                        contig = (len(masked) == masked[-1] - masked[0] + 1
                                  and slots == list(range(slots[0],
                                                          slots[0] + len(slots))))
                        if contig:
                            j0, sw = masked[0], len(masked)
                            nc.sync.dma_start(
                                out=mt[:, j0:j0 + sw, :],
                                in_=mask_t[slots[0]:slots[0] + sw].rearrange(
                                    "s p n -> p s n"))
                        else:
                            for i, j in enumerate(masked):
                                nc.sync.dma_start(out=mt[:, j, :],
                                                  in_=mask_t[slots[i]])
                        if len(masked) == w:
                            nc.vector.tensor_mul(
                                pT[:, :w, :], pT[:, :w, :], mt[:, :w, :])
                        else:
                            for j in masked:
                                nc.vector.tensor_mul(
                                    pT[:, j, :], pT[:, j, :], mt[:, j, :])
                    return pT

                def emit_final_recip(qs, qb):
                    dps = state[("dps", qs)]
                    c0, c1 = qb * 128, (qb + 1) * 128
                    recf = smp.tile([1, 128], F32, tag="recf", name="recf")
                    nc.vector.reciprocal(recf, dps[:, c0:c1])
                    recs = smp.tile([1, 128], BF16, tag="recs", name="recs")
                    nc.vector.tensor_copy(out=recs, in_=recf)
                    return recs

                def emit_final_qb_head(qs, qb, recs):
                    ot = state[("ot", qs)]
                    c0, c1 = qb * 128, (qb + 1) * 128
                    bps = syp.tile([128, 128], F32, tag="sps", name="bps")
                    nc.tensor.matmul(bps[:, :], ones_1[:, :], recs[:, :],
                                     start=True, stop=True)
                    rb = smp.tile([128, 128], BF16, tag="rb", name="rb")
                    nc.scalar.copy(out=rb, in_=bps)
                    for ft in range(4):
                        nc.vector.tensor_mul(ot[:, ft, c0:c1],
                                             ot[:, ft, c0:c1], rb)
                    for gh in range(2):
                        ypsg = ypp.tile([128, 4, 128], F32, tag="yp", name="ypsg")
                        for gi in range(4):
                            gt = gh * 4 + gi
                            for ft in range(4):
                                nc.tensor.matmul(
                                    ypsg[:, gi, :],
                                    wo_sb[:, ft, gt * 128:(gt + 1) * 128],
                                    ot[:, ft, c0:c1],
                                    start=(ft == 0),
                                    stop=(zero_bias and ft == 3))
                            if not zero_bias:
                                nc.tensor.matmul(
                                    ypsg[:, gi, :],
                                    byr_s[:, gt * 128:(gt + 1) * 128],
                                    ones_1[:, :], start=False, stop=True)
                        y_sbq = yop.tile([128, 4, 128], F32,
                                         name="ysq", tag="ysq")
                        if qb == 3 and gh == 1:
                            nc.vector.tensor_copy(out=y_sbq, in_=ypsg)
                        else:
                            nc.scalar.copy(out=y_sbq, in_=ypsg)
                        yq = (nc.sync, nc.scalar)[gh] if qb == 3 else \
                            (nc.sync, nc.gpsimd)[gh]
                        yq.dma_start(
                            out=y[gh * 512:(gh + 1) * 512,
                                  qs * 512 + c0:qs * 512 + c1].rearrange(
                                      "(a p) n -> p a n", p=128),
                            in_=y_sbq)

                def emit_attnv_job(job, pts):
                    qs, qb, groups = job
                    dps = state[("dps", qs)]
                    n = sum(len(g) for g in groups)
                    i = 0
                    for gi, grp in enumerate(groups):
                        for j, kt in enumerate(grp):
                            nc.tensor.matmul(
                                dps[:, qb * 128:(qb + 1) * 128],
                                ones_k[:, :], pts[gi][:, j, :],
                                start=(i == 0), stop=(i == n - 1))
                            i += 1
                    recs = None
                    if qs == QS - 1:
                        recs = emit_final_recip(qs, qb)
                    ops = opp.tile([128, 4, 128], F32, name="ops", tag="ops")
                    for dvt in range(4):
                        i = 0
                        for gi, grp in enumerate(groups):
                            for j, kt in enumerate(grp):
                                nc.tensor.matmul(
                                    ops[:, dvt, :],
                                    v_sb[:, kt, dvt * 128:(dvt + 1) * 128],
                                    pts[gi][:, j, :],
                                    start=(i == 0), stop=(i == n - 1))
                                i += 1
                    nc.vector.tensor_copy(
                        out=state[("ot", qs)][:, :, qb * 128:(qb + 1) * 128],
                        in_=ops)
                    if qs == QS - 1:
                        emit_final_qb_head(qs, qb, recs)

                def head_a(qs):
                    dps = state.pop(("dps", qs))
                    recip = smp.tile([1, 512], F32, tag="recf", name="recip")
                    nc.vector.reciprocal(recip, dps)
                    recs = smp.tile([1, 512], BF16, tag="recs", name="recs")
                    nc.vector.tensor_copy(out=recs, in_=recip)
                    state[("recs", qs)] = recs

                def head_b(qs):
                    ot = state.pop(("ot", qs))
                    recs = state.pop(("recs", qs))
                    bps = ypp.tile([128, 512], F32, tag="yp", name="bps")
                    nc.tensor.matmul(bps[:, :], ones_1[:, :], recs[:, :],
                                     start=True, stop=True)
                    rb = smp.tile([128, 512], F32, tag="rb", name="rb")
                    nc.scalar.copy(out=rb, in_=bps)
                    yqs = ([nc.sync, nc.gpsimd, nc.scalar, nc.sync]
                           if qs == QS - 1 else
                           [nc.gpsimd, nc.scalar, nc.gpsimd, nc.scalar])
                    for gp in range(4):
                        y_sb = yop.tile([128, 2, 512], F32,
                                        name="y_sb", tag="y_sb")
                        for gi in range(2):
                            gt = gp * 2 + gi
                            yps = ypp.tile([128, 512], F32, tag="yp", name="yps")
                            for ft in range(4):
                                nc.tensor.matmul(
                                    yps[:, :],
                                    wo_sb[:, ft, gt * 128:(gt + 1) * 128],
                                    ot[:, ft, :], start=(ft == 0), stop=(ft == 3))
                            ym = yop.tile([128, 512], F32, name="ym", tag="ym")
                            nc.vector.tensor_mul(ym, yps, rb)
                            if zero_bias:
                                nc.scalar.copy(out=y_sb[:, gi, :], in_=ym)
                            else:
                                nc.scalar.activation(
                                    out=y_sb[:, gi, :], in_=ym,
                                    func=mybir.ActivationFunctionType.Identity,
                                    bias=by_s[:, gt:gt + 1])
                        yqs[gp].dma_start(
                            out=y[gp * 256:(gp + 1) * 256,
                                  qs * 512:(qs + 1) * 512].rearrange(
                                      "(a p) n -> p a n", p=128),
                            in_=y_sb)

                heads = []
                HEAD_STAGES = (head_a, head_b)

                def step_heads():
                    for h in list(heads):
                        HEAD_STAGES[h[1]](h[0])
                        h[1] += 1
                        if h[1] == len(HEAD_STAGES):
                            heads.remove(h)

                def retire(job, pts):
                    emit_attnv_job(job, pts)
                    qs, qb = job[0], job[1]
                    if qb == 3:
                        if qs == QS - 1:
                            state.pop(("ot", qs), None)
                            state.pop(("dps", qs), None)
                        else:
                            heads.append([qs, 0])
                    step_heads()

                prev = None
                for job in jobs:
                    qs, qb, groups = job
                    if ("ot", qs) not in state:
                        state[("ot", qs)] = otp.tile(
                            [128, 4, 512], BF16, name="ot", tag="ot")
                        state[("dps", qs)] = dnp.tile(
                            [1, 512], F32, name="dps", tag="dps")
                    pts = [emit_scores_grp(qs, qb, grp) for grp in groups]
                    if prev is not None:
                        retire(*prev)
                    prev = (job, pts)
                retire(*prev)
                while heads:
                    step_heads()

    nc.compile()
    return nc


def _prep_mask(mask):
    """Compile-time 128x128 tile analysis for ONE stream's mask."""
    mt = np.ascontiguousarray(np.asarray(mask).T)
    kept = {}
    needs_mask = set()
    slot_of = {}
    slots = []  # (qb, kt)
    for g in range(QB):
        klist = []
        for kt in range(KT):
            sub = mt[kt * 128:(kt + 1) * 128, g * 128:(g + 1) * 128]
            if not sub.any():
                continue
            klist.append(kt)
            if not sub.all():
                needs_mask.add((g, kt))
                slot_of[(g, kt)] = len(slots)
                slots.append((g, kt))
        kept[g] = klist
    n_slots = max(1, len(slots))
    md = np.zeros((n_slots, 128, 128), BF)
    for i, (g, kt) in enumerate(slots):
        md[i] = mt[kt * 128:(kt + 1) * 128,
                   g * 128:(g + 1) * 128].astype(BF)
    return kept, needs_mask, slot_of, n_slots, md


def _split8(a):
    h = a.astype(H8)
    l = (a - h.astype(np.float32)).astype(E5)
    return h, l


def kernel(q_real, q_imag, k_real, k_imag, v_real, v_imag,
           W_qkv, b_qkv, W_out, b_out, mask_real, mask_imag, _trace=False):
    global LAST_RESULTS, LAST_PROGRAMS
    args = [np.asarray(a) for a in (q_real, q_imag, k_real, k_imag, v_real, v_imag)]
    W_qkv = np.asarray(W_qkv, np.float32)
    b_qkv = np.asarray(b_qkv, np.float32)
    W_out = np.asarray(W_out, np.float32)
    b_out = np.asarray(b_out, np.float32)

    zb = bool(not b_qkv.any() and not b_out.any())
    preps = [_prep_mask(mask_real), _prep_mask(mask_imag)]
    programs = [build_program(*p[:4], zero_bias=zb) for p in preps]

    # x^T per batch, c-tiled hi/lo: [CT, 128, L] e4m3 + e5m2
    x8_ts, xl_ts = [], []
    for b in range(B):
        xb = np.concatenate([a[b] for a in args], axis=1)          # [L, 6D]
        xt = np.ascontiguousarray(xb.T.astype(np.float32))          # [6D, L]
        xh, xl = _split8(xt)
        x8_ts.append(np.ascontiguousarray(xh.reshape(CT, 128, L)))
        xl_ts.append(np.ascontiguousarray(xl.reshape(CT, 128, L)))

    W6T = W_qkv.T  # [c, f]
    W2T = W_out.T  # [f=2D, g=2D]
    stream_inputs = []
    for s in range(2):
        wq = W6T[:, _Q_OFF[s]:_Q_OFF[s] + D].reshape(CT, 128, 512)
        wk = W6T[:, _K_OFF[s]:_K_OFF[s] + D].reshape(CT, 128, 512)
        wv = W6T[:, _V_OFF[s]:_V_OFF[s] + D].reshape(CT, 128, 512)
        wqk = np.ascontiguousarray(np.stack([wq, wk], axis=2))     # [CT,128,2,512]
        wqkh, wqkl = _split8(wqk)
        wvh, wvl = _split8(np.ascontiguousarray(wv))
        bq = b_qkv[_Q_OFF[s]:_Q_OFF[s] + D].reshape(4, 128).T
        bk = b_qkv[_K_OFF[s]:_K_OFF[s] + D].reshape(4, 128).T
        b_qks = np.ascontiguousarray(
            np.concatenate([bq, bk], axis=1), dtype=np.float32)    # [128, 8]
        w_os = np.ascontiguousarray(
            W2T[s * D:(s + 1) * D, :].reshape(4, 128, 2 * D).astype(BF))
        if s == 0:
            b_v_cat = np.concatenate([b_qkv[_V_OFF[0]:_V_OFF[0] + D],
                                      b_qkv[_V_OFF[1]:_V_OFF[1] + D]])
            b_eff = (W_out @ b_v_cat + b_out).astype(np.float32)
            b_ys = np.ascontiguousarray(b_eff.reshape(8, 128).T)
        else:
            b_ys = np.zeros((128, 8), np.float32)
        stream_inputs.append(dict(
            wqkh=np.ascontiguousarray(wqkh), wqkl=np.ascontiguousarray(wqkl),
            wvh=np.ascontiguousarray(wvh), wvl=np.ascontiguousarray(wvl),
            b_qks=b_qks, w_os=w_os, b_ys=b_ys))

    LAST_RESULTS = []
    LAST_PROGRAMS = programs
    stream_res = []
    for s in range(2):
        si = stream_inputs[s]
        in_maps = []
        for b in range(B):
            in_maps.append({
                "x8_t": x8_ts[b], "xl_t": xl_ts[b],
                "wqkh_t": si["wqkh"], "wqkl_t": si["wqkl"],
                "wvh_t": si["wvh"], "wvl_t": si["wvl"],
                "w_o": si["w_os"], "b_qk": si["b_qks"], "b_y": si["b_ys"],
                "b_yr": np.ascontiguousarray(
                    si["b_ys"].T.reshape(1, 2 * D).astype(BF)),
                "mask_t": preps[s][4],
                "ones_a": np.ones((128, 1), BF),
                "ones_b": np.ones((1, 128), BF),
            })
        res = run_bass_kernel_spmd(programs[s], in_maps,
                                   core_ids=[4 * s + b for b in range(B)],
                                   trace=_trace)
        LAST_RESULTS.append(res)
        stream_res.append(res)

    out_real = np.empty((B, L, D), np.float32)
    out_imag = np.empty((B, L, D), np.float32)
    for b in range(B):
        yt = stream_res[0].results[b]["y"] + stream_res[1].results[b]["y"]
        yb = yt.T                                                   # [L, 2D]
        out_real[b] = yb[:, :D]
        out_imag[b] = yb[:, D:]
    return out_real, out_imag


# revision 17
# speedup vs baseline: 1.2437x; 1.0467x over previous
"""Bass/Trainium2 kernel for nn_BasicQuantumAttention (B=4, L=2048, d=512, 8 cores).

Sharding: core (b, s) = batch b, stream s (real/imag); one program per
stream (each stream's own block-sparse keep-set; ~52/60 kept 128x128
tiles vs 79 for the union). Each core:
  - projects x[b] -> qT, kT (layout [d, L]) and v (layout [L, d]), all
    SBUF-resident. The projection runs as fp8 DoubleRow matmuls with
    hi/lo error compensation: every operand A is split into
    A_hi = e4m3(A) and A_lo = e5m2(A - A_hi), and A@B is computed as
    three DoubleRow pass chains (Ah@Bh, Ah@Bl, Al@Bh). DoubleRow
    contracts two 128-K slabs per instruction, so the three passes cost
    0.75x the bf16 cycles while matching bf16 accuracy (the dropped
    Al@Bl term is ~2^-8 relative). q/k are evicted from PSUM as
    (e4m3 hi, e5m2 lo) pairs so the score matmuls use the same scheme;
    v is evicted bf16 for the (bf16) attnV matmuls.
  - block-sparse masked attention with compile-time tile skipping at
    128x128 granularity on this stream's mask; scores are fp8 tri-term
    DoubleRow, exp/mask/attnV as in the bf16 kernel.
  - partial out-projection y^T_part = W_out^T[stream rows].T @ O_norm^T
    (bf16).
Host sums the two partial y^T per batch and untransposes.
"""
import sys

sys.path.insert(0, "/opt/trn_rl_repo")

import numpy as np
import ml_dtypes

import concourse.bass as bass
import concourse.tile as tile
from concourse import bacc, mybir
from concourse.bass_utils import run_bass_kernel_spmd

B, L, D = 4, 2048, 512
C6 = 6 * D            # 3072 input features
CT = C6 // 128        # 24 contraction tiles
QS = L // 512         # 4 query slices of 512 (normalization/out-proj grain)
QB = L // 128         # 16 query blocks of 128 (attention grain)
KT = L // 128         # 16 key tiles of 128
F32 = mybir.dt.float32
BF16 = mybir.dt.bfloat16
F8H = mybir.dt.float8e4
F8L = mybir.dt.float8e5
DR = mybir.MatmulPerfMode.DoubleRow
SCALE = float(D) ** -0.5
BF = ml_dtypes.bfloat16
H8 = ml_dtypes.float8_e4m3
E5 = ml_dtypes.float8_e5m2

# feature offsets inside qkv = [q_r q_i k_r k_i v_r v_i] (each D wide)
_Q_OFF = {0: 0 * D, 1: 1 * D}
_K_OFF = {0: 2 * D, 1: 3 * D}
_V_OFF = {0: 4 * D, 1: 5 * D}

LAST_RESULTS = None   # list of per-stream BassKernelResults
LAST_PROGRAMS = None  # list of per-stream compiled Bacc programs


def build_program(kept, needs_mask, slot_of, n_slots, zero_bias=False):
    """kept: {qb_global: [kt,...]} keep lists at 128x128 granularity for
    THIS stream; needs_mask: set[(qb,kt)]; slot_of: {(qb,kt): slot}."""
    nc = bacc.Bacc(None, target_bir_lowering=False, debug=False)

    x8_t = nc.dram_tensor("x8_t", [CT, 128, L], F8H, kind="ExternalInput")
    xl_t = nc.dram_tensor("xl_t", [CT, 128, L], F8L, kind="ExternalInput")
    wqkh_t = nc.dram_tensor("wqkh_t", [CT, 128, 2, 512], F8H, kind="ExternalInput")
    wqkl_t = nc.dram_tensor("wqkl_t", [CT, 128, 2, 512], F8L, kind="ExternalInput")
    wvh_t = nc.dram_tensor("wvh_t", [CT, 128, 512], F8H, kind="ExternalInput")
    wvl_t = nc.dram_tensor("wvl_t", [CT, 128, 512], F8L, kind="ExternalInput")
    w_o = nc.dram_tensor("w_o", [4, 128, 2 * D], BF16, kind="ExternalInput")
    woh_t = nc.dram_tensor("woh_t", [4, 128, 2 * D], F8H, kind="ExternalInput")
    wol_t = nc.dram_tensor("wol_t", [4, 128, 2 * D], F8L, kind="ExternalInput")
    b_qk = nc.dram_tensor("b_qk", [128, 8], F32, kind="ExternalInput")
    b_y = nc.dram_tensor("b_y", [128, 8], F32, kind="ExternalInput")
    mask_t = nc.dram_tensor("mask_t", [n_slots, 128, 128], BF16, kind="ExternalInput")
    ones_a = nc.dram_tensor("ones_a", [128, 1], BF16, kind="ExternalInput")
    ones_b = nc.dram_tensor("ones_b", [1, 128], BF16, kind="ExternalInput")
    b_yr = nc.dram_tensor("b_yr", [1, 2 * D], BF16, kind="ExternalInput")
    y = nc.dram_tensor("y", [2 * D, L], F32, kind="ExternalOutput")

    with tile.TileContext(nc) as tc, \
         nc.allow_low_precision(reason="fp8 hi/lo compensated matmuls"):
        with tc.tile_pool(name="consts", bufs=1) as consts, \
             tc.tile_pool(name="kqv", bufs=1) as kqv:
            ones_k = consts.tile([128, 1], BF16)
            ones_1 = consts.tile([1, 128], BF16)
            bqk_s = consts.tile([128, 8], F32)
            by_s = consts.tile([128, 8], F32)
            byr_s = consts.tile([1, 2 * D], BF16)
            if zero_bias:
                woh_sb = consts.tile([128, 4, 2 * D], F8H)
                wol_sb = consts.tile([128, 4, 2 * D], F8L)
            else:
                wo_sb = consts.tile([128, 4, 2 * D], BF16)

            qh_sb = kqv.tile([128, 4, L], F8H)
            ql_sb = kqv.tile([128, 4, L], F8L)
            kh_sb = kqv.tile([128, 4, L], F8H)
            kl_sb = kqv.tile([128, 4, L], F8L)
            v_sb = kqv.tile([128, KT, 512], BF16)

            # ---------------- projection phase ----------------
            with tc.tile_pool(name="wc", bufs=1) as wcp, \
                 tc.tile_pool(name="xin", bufs=2) as xp, \
                 tc.tile_pool(name="ev", bufs=4) as evp, \
                 tc.tile_pool(name="pp", bufs=8, space="PSUM") as pp:
                wqkh_sb = wcp.tile([128, CT, 2, 512], F8H)
                wqkl_sb = wcp.tile([128, CT, 2, 512], F8L)
                wvh_sb = wcp.tile([128, CT, 512], F8H)
                wvl_sb = wcp.tile([128, CT, 512], F8L)

                x_tiles = {}

                def load_x(qs_):
                    xh = xp.tile([128, CT, 512], F8H, name=f"x8{qs_}", tag="x8")
                    xl = xp.tile([128, CT, 512], F8L, name=f"xl{qs_}", tag="xl")
                    sl = slice(qs_ * 512, (qs_ + 1) * 512)
                    nc.sync.dma_start(
                        out=xh, in_=x8_t[:, :, sl].rearrange("ct p n -> p ct n"))
                    nc.sync.dma_start(
                        out=xl, in_=xl_t[:, :, sl].rearrange("ct p n -> p ct n"))
                    x_tiles[qs_] = (xh, xl)

                # PE p-state warm-up: burn the 0.65->2.4GHz ramp on dummy
                # matmuls while the first input DMAs are in flight
                warm = consts.tile([128, 128], BF16)
                nc.vector.memset(warm, 0.0)
                wps = pp.tile([128, 512], F32, name="wps", tag="ps")
                for _ in range(8):
                    nc.tensor.matmul(wps[:, 0:128], warm[:, :], warm[:, :],
                                     start=True, stop=True)

                x0h = xp.tile([128, CT, 512], F8H, name="x80", tag="x8")
                x0l = xp.tile([128, CT, 512], F8L, name="xl0", tag="xl")
                # qs=0 runs wave1 = q+k (8 chains, consuming wqk hi/lo + x
                # hi/lo at ~1.16us/ct delivered vs 1.28us/ct consumed) then
                # wave2 = v (wv hi/lo, delivered during wave1's tail). The
                # DMA engines are one serialized resource, so the sync
                # stream is ordered exactly in consumption order; only the
                # first x8 piece + consts ride gpsimd for issue parallelism.
                nc.gpsimd.dma_start(out=x0h[:, 0:2, :],
                                    in_=x8_t[0:2, :, 0:512].rearrange(
                                        "ct p n -> p ct n"))
                nc.sync.dma_start(
                    out=wqkh_sb[:, 0:2, :, :],
                    in_=wqkh_t[0:2].rearrange("ct p f d -> p ct f d"))
                nc.sync.dma_start(
                    out=wqkl_sb[:, 0:2, :, :],
                    in_=wqkl_t[0:2].rearrange("ct p f d -> p ct f d"))
                nc.sync.dma_start(out=x0l[:, 0:2, :],
                                  in_=xl_t[0:2, :, 0:512].rearrange(
                                      "ct p n -> p ct n"))
                # consts trail on gpsimd (nothing needs them until the first
                # evictions ~15us in)
                nc.gpsimd.dma_start(out=ones_k, in_=ones_a[:, :])
                nc.gpsimd.dma_start(out=ones_1, in_=ones_b[:, :])
                nc.gpsimd.dma_start(out=bqk_s, in_=b_qk[:, :])
                nc.gpsimd.dma_start(out=by_s, in_=b_y[:, :])
                nc.gpsimd.dma_start(out=byr_s, in_=b_yr[:, :])
                # pre-warm the exp activation table while PE projects
                scrap = consts.tile([128, 8], BF16)
                nc.scalar.activation(out=scrap, in_=bqk_s,
                                     func=mybir.ActivationFunctionType.Exp)
                ct_groups = [list(range(c, min(c + 3, CT)))
                             for c in range(2, CT, 3)]
                for grp_ in ct_groups:
                    c0, cn = grp_[0], len(grp_)
                    nc.sync.dma_start(
                        out=wqkh_sb[:, c0:c0 + cn, :, :],
                        in_=wqkh_t[c0:c0 + cn].rearrange("ct p f d -> p ct f d"))
                    nc.sync.dma_start(
                        out=wqkl_sb[:, c0:c0 + cn, :, :],
                        in_=wqkl_t[c0:c0 + cn].rearrange("ct p f d -> p ct f d"))
                    nc.sync.dma_start(
                        out=x0h[:, c0:c0 + cn, :],
                        in_=x8_t[c0:c0 + cn, :, 0:512].rearrange(
                            "ct p n -> p ct n"))
                    nc.sync.dma_start(
                        out=x0l[:, c0:c0 + cn, :],
                        in_=xl_t[c0:c0 + cn, :, 0:512].rearrange(
                            "ct p n -> p ct n"))
                # wv hi/lo stream for wave2, in consumption order
                for c0 in range(0, CT, 3):
                    cn = min(3, CT - c0)
                    nc.sync.dma_start(
                        out=wvh_sb[:, c0:c0 + cn, :],
                        in_=wvh_t[c0:c0 + cn].rearrange("ct p d -> p ct d"))
                    nc.sync.dma_start(
                        out=wvl_sb[:, c0:c0 + cn, :],
                        in_=wvl_t[c0:c0 + cn].rearrange("ct p d -> p ct d"))
                x_tiles[0] = (x0h, x0l)
                load_x(1)

                def evict(kind, ft, ps, qs_):
                    if kind == "v":
                        # v bias is folded into b_y on the host (as in the
                        # bf16 kernel), so v eviction is always a plain copy
                        nc.scalar.copy(out=v_sb[:, qs_ * 4 + ft, :], in_=ps)
                        return
                    hi, lo = (qh_sb, ql_sb) if kind == "q" else (kh_sb, kl_sb)
                    bi = ft if kind == "q" else 4 + ft
                    sl = slice(qs_ * 512, (qs_ + 1) * 512)
                    if zero_bias:
                        nc.scalar.copy(out=hi[:, ft, sl], in_=ps)
                        nc.vector.tensor_sub(lo[:, ft, sl], ps, hi[:, ft, sl])
                    else:
                        tmp = evp.tile([128, 512], BF16, name="evt", tag="evt")
                        nc.scalar.activation(
                            out=tmp, in_=ps,
                            func=mybir.ActivationFunctionType.Identity,
                            bias=bqk_s[:, bi:bi + 1])
                        nc.scalar.copy(out=hi[:, ft, sl], in_=tmp)
                        nc.vector.tensor_sub(lo[:, ft, sl], tmp, hi[:, ft, sl])

                def mm_steps(kind, ft, c, xt):
                    """The 3 DoubleRow (lhsT, rhs) pairs for ct-pair c."""
                    xh, xl = xt
                    cp = slice(c, c + 2)
                    fsl = slice(ft * 128, (ft + 1) * 128)
                    if kind == "v":
                        return [(xh[:, cp, fsl], wvh_sb[:, cp, :]),
                                (xh[:, cp, fsl], wvl_sb[:, cp, :]),
                                (xl[:, cp, fsl], wvh_sb[:, cp, :])]
                    fc = 0 if kind == "q" else 1
                    return [(wqkh_sb[:, cp, fc, fsl], xh[:, cp, :]),
                            (wqkl_sb[:, cp, fc, fsl], xh[:, cp, :]),
                            (wqkh_sb[:, cp, fc, fsl], xl[:, cp, :])]

                NP = 3 * (CT // 2)  # matmuls per chain

                # qs=0: ct-pair-major waves so PE consumption tracks DMA
                # delivery; wave1 = q+k (needs wqk+x, 8 PSUM banks),
                # wave2 = v (wv lands during wave1)
                x0 = x_tiles.pop(0)
                waves = [[("q", ft) for ft in range(4)] +
                         [("k", ft) for ft in range(4)],
                         [("v", nt) for nt in range(4)]]
                for wave in waves:
                    pss = {u: pp.tile([128, 512], F32, name=f"ps{u[0]}{u[1]}",
                                      tag="ps") for u in wave}
                    cnt = {u: 0 for u in wave}
                    for c in range(0, CT, 2):
                        for u in wave:
                            for lhsT, rhs in mm_steps(u[0], u[1], c, x0):
                                nc.tensor.matmul(
                                    pss[u][:, :], lhsT, rhs,
                                    start=(cnt[u] == 0),
                                    stop=(cnt[u] == NP - 1), perf_mode=DR)
                                cnt[u] += 1
                    for u in wave:
                        evict(u[0], u[1], pss[u], 0)

                for qs in range(1, QS):
                    if qs + 1 < QS:
                        load_x(qs + 1)
                    x_qs = x_tiles.pop(qs)
                    for kind in ("q", "k", "v"):
                        for ft in range(4):
                            ps = pp.tile([128, 512], F32, name="ps", tag="ps")
                            i = 0
                            for c in range(0, CT, 2):
                                for lhsT, rhs in mm_steps(kind, ft, c, x_qs):
                                    nc.tensor.matmul(
                                        ps[:, :], lhsT, rhs,
                                        start=(i == 0), stop=(i == NP - 1),
                                        perf_mode=DR)
                                    i += 1
                            evict(kind, ft, ps, qs)

            # ---------------- attention + out-projection ----------------
            jobs = []   # (qs, qb, [groups of up to 4 kt])
            for qs in range(QS):
                # biggest jobs first within each qs: the overall last job is
                # then small, shrinking the end-of-program exp/attnV drain
                order = sorted(range(4),
                               key=lambda qb: -len(kept[qs * 4 + qb]))
                for qb in order:
                    klist = kept[qs * 4 + qb]
                    jobs.append((qs, qb,
                                 [klist[i:i + 4]
                                  for i in range(0, len(klist), 4)]))

            with tc.tile_pool(name="sy", bufs=3, space="PSUM") as syp, \
                 tc.tile_pool(name="op", bufs=2, space="PSUM") as opp, \
                 tc.tile_pool(name="dn", bufs=1, space="PSUM") as dnp, \
                 tc.tile_pool(name="yp", bufs=2, space="PSUM") as ypp, \
                 tc.tile_pool(name="pt", bufs=8) as ptp, \
                 tc.tile_pool(name="mk", bufs=8) as mkp, \
                 tc.tile_pool(name="ot", bufs=3) as otp, \
                 tc.tile_pool(name="ot8", bufs=2) as ot8p, \
                 tc.tile_pool(name="sm", bufs=2) as smp, \
                 tc.tile_pool(name="yo", bufs=4) as yop:
                if zero_bias:
                    nc.scalar.dma_start(
                        out=woh_sb, in_=woh_t.rearrange("ft p g -> p ft g"))
                    nc.scalar.dma_start(
                        out=wol_sb, in_=wol_t.rearrange("ft p g -> p ft g"))
                else:
                    nc.scalar.dma_start(
                        out=wo_sb, in_=w_o.rearrange("ft p g -> p ft g"))

                state = {}   # per-qs tiles: ot, dps

                def emit_scores_grp(qs, qb, grp):
                    g = qs * 4 + qb
                    w = len(grp)
                    gsl = slice(g * 128, (g + 1) * 128)
                    sps = syp.tile([128, 4, 128], F32, name="sps", tag="sps")
                    for j, kt in enumerate(grp):
                        ksl = slice(kt * 128, (kt + 1) * 128)
                        steps = []
                        for dt in (0, 2):
                            steps.append((kh_sb[:, dt:dt + 2, ksl],
                                          qh_sb[:, dt:dt + 2, gsl]))
                        for dt in (0, 2):
                            steps.append((kh_sb[:, dt:dt + 2, ksl],
                                          ql_sb[:, dt:dt + 2, gsl]))
                        for dt in (0, 2):
                            steps.append((kl_sb[:, dt:dt + 2, ksl],
                                          qh_sb[:, dt:dt + 2, gsl]))
                        for i, (lhsT, rhs) in enumerate(steps):
                            nc.tensor.matmul(
                                sps[:, j, :], lhsT, rhs,
                                start=(i == 0), stop=(i == len(steps) - 1),
                                perf_mode=DR)
                    pT = ptp.tile([128, 4, 128], BF16, name="pT", tag="pT")
                    nc.scalar.activation(
                        out=pT[:, :w, :], in_=sps[:, :w, :],
                        func=mybir.ActivationFunctionType.Exp, scale=SCALE)
                    masked = [j for j, kt in enumerate(grp)
                              if (g, kt) in needs_mask]
                    if masked:
                        mt = mkp.tile([128, 4, 128], BF16, name="mt", tag="mt")
                        slots = [slot_of[(g, grp[j])] for j in masked]
                        contig = (len(masked) == masked[-1] - masked[0] + 1
                                  and slots == list(range(slots[0],
                                                          slots[0] + len(slots))))
                        if contig:
                            j0, sw = masked[0], len(masked)
                            nc.sync.dma_start(
                                out=mt[:, j0:j0 + sw, :],
                                in_=mask_t[slots[0]:slots[0] + sw].rearrange(
                                    "s p n -> p s n"))
                        else:
                            for i, j in enumerate(masked):
                                nc.sync.dma_start(out=mt[:, j, :],
                                                  in_=mask_t[slots[i]])
                        if len(masked) == w:
                            nc.vector.tensor_mul(
                                pT[:, :w, :], pT[:, :w, :], mt[:, :w, :])
                        else:
                            for j in masked:
                                nc.vector.tensor_mul(
                                    pT[:, j, :], pT[:, j, :], mt[:, j, :])
                    return pT

                def emit_final_recip(qs, qb):
                    dps = state[("dps", qs)]
                    c0, c1 = qb * 128, (qb + 1) * 128
                    recf = smp.tile([1, 128], F32, tag="recf", name="recf")
                    nc.vector.reciprocal(recf, dps[:, c0:c1])
                    recs = smp.tile([1, 128], BF16, tag="recs", name="recs")
                    nc.vector.tensor_copy(out=recs, in_=recf)
                    return recs

                def emit_final_qb_head(qs, qb, recs, last):
                    ot = state[("ot", qs)]
                    c0, c1 = qb * 128, (qb + 1) * 128
                    bps = syp.tile([128, 128], F32, tag="sps", name="bps")
                    nc.tensor.matmul(bps[:, :], ones_1[:, :], recs[:, :],
                                     start=True, stop=True)
                    rb = smp.tile([128, 128], BF16, tag="rb", name="rb")
                    nc.scalar.copy(out=rb, in_=bps)
                    for ft in range(4):
                        nc.vector.tensor_mul(ot[:, ft, c0:c1],
                                             ot[:, ft, c0:c1], rb)
                    if zero_bias:
                        oth = ot8p.tile([128, 4, 128], F8H, name="othq",
                                        tag="othq")
                        otl = ot8p.tile([128, 4, 128], F8L, name="otlq",
                                        tag="otlq")
                        nc.scalar.copy(out=oth, in_=ot[:, :, c0:c1])
                        nc.vector.tensor_sub(otl, ot[:, :, c0:c1], oth)
                    for gh in range(2):
                        ypsg = ypp.tile([128, 4, 128], F32, tag="yp", name="ypsg")
                        for gi in range(4):
                            gt = gh * 4 + gi
                            gsl = slice(gt * 128, (gt + 1) * 128)
                            if zero_bias:
                                steps = []
                                for ft in (0, 2):
                                    steps.append((woh_sb[:, ft:ft + 2, gsl],
                                                  oth[:, ft:ft + 2, :]))
                                for ft in (0, 2):
                                    steps.append((woh_sb[:, ft:ft + 2, gsl],
                                                  otl[:, ft:ft + 2, :]))
                                for ft in (0, 2):
                                    steps.append((wol_sb[:, ft:ft + 2, gsl],
                                                  oth[:, ft:ft + 2, :]))
                                for i, (lh, rh) in enumerate(steps):
                                    nc.tensor.matmul(
                                        ypsg[:, gi, :], lh, rh,
                                        start=(i == 0), stop=(i == 5),
                                        perf_mode=DR)
                            else:
                                for ft in range(4):
                                    nc.tensor.matmul(
                                        ypsg[:, gi, :],
                                        wo_sb[:, ft, gsl],
                                        ot[:, ft, c0:c1],
                                        start=(ft == 0), stop=False)
                                nc.tensor.matmul(
                                    ypsg[:, gi, :],
                                    byr_s[:, gsl],
                                    ones_1[:, :], start=False, stop=True)
                        y_sbq = yop.tile([128, 4, 128], F32,
                                         name="ysq", tag="ysq")
                        if last and gh == 1:
                            nc.vector.tensor_copy(out=y_sbq, in_=ypsg)
                        else:
                            nc.scalar.copy(out=y_sbq, in_=ypsg)
                        yq = (nc.sync, nc.scalar)[gh] if last else \
                            (nc.sync, nc.gpsimd)[gh]
                        yq.dma_start(
                            out=y[gh * 512:(gh + 1) * 512,
                                  qs * 512 + c0:qs * 512 + c1].rearrange(
                                      "(a p) n -> p a n", p=128),
                            in_=y_sbq)

                def emit_attnv_job(job, pts, last):
                    qs, qb, groups = job
                    dps = state[("dps", qs)]
                    n = sum(len(g) for g in groups)
                    i = 0
                    for gi, grp in enumerate(groups):
                        for j, kt in enumerate(grp):
                            nc.tensor.matmul(
                                dps[:, qb * 128:(qb + 1) * 128],
                                ones_k[:, :], pts[gi][:, j, :],
                                start=(i == 0), stop=(i == n - 1))
                            i += 1
                    recs = None
                    if qs == QS - 1:
                        recs = emit_final_recip(qs, qb)
                    ops = opp.tile([128, 4, 128], F32, name="ops", tag="ops")
                    for dvt in range(4):
                        i = 0
                        for gi, grp in enumerate(groups):
                            for j, kt in enumerate(grp):
                                nc.tensor.matmul(
                                    ops[:, dvt, :],
                                    v_sb[:, kt, dvt * 128:(dvt + 1) * 128],
                                    pts[gi][:, j, :],
                                    start=(i == 0), stop=(i == n - 1))
                                i += 1
                    nc.vector.tensor_copy(
                        out=state[("ot", qs)][:, :, qb * 128:(qb + 1) * 128],
                        in_=ops)
                    if qs == QS - 1:
                        emit_final_qb_head(qs, qb, recs, last)

                def head_a(qs):
                    dps = state.pop(("dps", qs))
                    recip = smp.tile([1, 512], F32, tag="recf", name="recip")
                    nc.vector.reciprocal(recip, dps)
                    recs = smp.tile([1, 512], BF16, tag="recs", name="recs")
                    nc.vector.tensor_copy(out=recs, in_=recip)
                    state[("recs", qs)] = recs

                def head_b(qs):
                    ot = state.pop(("ot", qs))
                    recs = state.pop(("recs", qs))
                    bps = ypp.tile([128, 512], F32, tag="yp", name="bps")
                    nc.tensor.matmul(bps[:, :], ones_1[:, :], recs[:, :],
                                     start=True, stop=True)
                    rb = smp.tile([128, 512], BF16, tag="rb", name="rb")
                    nc.scalar.copy(out=rb, in_=bps)
                    if zero_bias:
                        # normalize ot in place, then split hi/lo for the
                        # DoubleRow out-projection
                        for ft in range(4):
                            nc.vector.tensor_mul(ot[:, ft, :], ot[:, ft, :], rb)
                        oth = ot8p.tile([128, 4, 512], F8H, name="oth",
                                        tag="oth")
                        otl = ot8p.tile([128, 4, 512], F8L, name="otl",
                                        tag="otl")
                        nc.scalar.copy(out=oth, in_=ot)
                        nc.vector.tensor_sub(otl, ot, oth)
                    yqs = ([nc.sync, nc.gpsimd, nc.scalar, nc.sync]
                           if qs == QS - 1 else
                           [nc.gpsimd, nc.scalar, nc.gpsimd, nc.scalar])
                    for gp in range(4):
                        y_sb = yop.tile([128, 2, 512], F32,
                                        name="y_sb", tag="y_sb")
                        for gi in range(2):
                            gt = gp * 2 + gi
                            gsl = slice(gt * 128, (gt + 1) * 128)
                            yps = ypp.tile([128, 512], F32, tag="yp", name="yps")
                            if zero_bias:
                                steps = []
                                for ft in (0, 2):
                                    steps.append((woh_sb[:, ft:ft + 2, gsl],
                                                  oth[:, ft:ft + 2, :]))
                                for ft in (0, 2):
                                    steps.append((woh_sb[:, ft:ft + 2, gsl],
                                                  otl[:, ft:ft + 2, :]))
                                for ft in (0, 2):
                                    steps.append((wol_sb[:, ft:ft + 2, gsl],
                                                  oth[:, ft:ft + 2, :]))
                                for i, (lh, rh) in enumerate(steps):
                                    nc.tensor.matmul(
                                        yps[:, :], lh, rh,
                                        start=(i == 0), stop=(i == 5),
                                        perf_mode=DR)
                                nc.scalar.copy(out=y_sb[:, gi, :], in_=yps)
                            else:
                                for ft in range(4):
                                    nc.tensor.matmul(
                                        yps[:, :],
                                        wo_sb[:, ft, gsl],
                                        ot[:, ft, :], start=(ft == 0),
                                        stop=(ft == 3))
                                ym = yop.tile([128, 512], F32, name="ym",
                                              tag="ym")
                                nc.vector.tensor_mul(ym, yps, rb)
                                nc.scalar.activation(
                                    out=y_sb[:, gi, :], in_=ym,
                                    func=mybir.ActivationFunctionType.Identity,
                                    bias=by_s[:, gt:gt + 1])
                        yqs[gp].dma_start(
                            out=y[gp * 256:(gp + 1) * 256,
                                  qs * 512:(qs + 1) * 512].rearrange(
                                      "(a p) n -> p a n", p=128),
                            in_=y_sb)

                heads = []
                HEAD_STAGES = (head_a, head_b)

                def step_heads():
                    for h in list(heads):
                        HEAD_STAGES[h[1]](h[0])
                        h[1] += 1
                        if h[1] == len(HEAD_STAGES):
                            heads.remove(h)

                done_cnt = {}

                def retire(job, pts, last=False):
                    emit_attnv_job(job, pts, last)
                    qs = job[0]
                    done_cnt[qs] = done_cnt.get(qs, 0) + 1
                    if done_cnt[qs] == 4:
                        if qs == QS - 1:
                            state.pop(("ot", qs), None)
                            state.pop(("dps", qs), None)
                        else:
                            heads.append([qs, 0])
                    step_heads()

                prev = None
                for job in jobs:
                    qs, qb, groups = job
                    if ("ot", qs) not in state:
                        state[("ot", qs)] = otp.tile(
                            [128, 4, 512], BF16, name="ot", tag="ot")
                        state[("dps", qs)] = dnp.tile(
                            [1, 512], F32, name="dps", tag="dps")
                    pts = [emit_scores_grp(qs, qb, grp) for grp in groups]
                    if prev is not None:
                        retire(*prev)
                    prev = (job, pts)
                retire(*prev, last=True)
                while heads:
                    step_heads()

    nc.compile()
    return nc


def _prep_mask(mask):
    """Compile-time 128x128 tile analysis for ONE stream's mask."""
    mt = np.ascontiguousarray(np.asarray(mask).T)
    kept = {}
    needs_mask = set()
    slot_of = {}
    slots = []  # (qb, kt)
    for g in range(QB):
        klist = []
        for kt in range(KT):
            sub = mt[kt * 128:(kt + 1) * 128, g * 128:(g + 1) * 128]
            if not sub.any():
                continue
            klist.append(kt)
            if not sub.all():
                needs_mask.add((g, kt))
                slot_of[(g, kt)] = len(slots)
                slots.append((g, kt))
        kept[g] = klist
    n_slots = max(1, len(slots))
    md = np.zeros((n_slots, 128, 128), BF)
    for i, (g, kt) in enumerate(slots):
        md[i] = mt[kt * 128:(kt + 1) * 128,
                   g * 128:(g + 1) * 128].astype(BF)
    return kept, needs_mask, slot_of, n_slots, md


def _split8(a):
    h = a.astype(H8)
    l = (a - h.astype(np.float32)).astype(E5)
    return h, l


def kernel(q_real, q_imag, k_real, k_imag, v_real, v_imag,
           W_qkv, b_qkv, W_out, b_out, mask_real, mask_imag, _trace=False):
    global LAST_RESULTS, LAST_PROGRAMS
    args = [np.asarray(a) for a in (q_real, q_imag, k_real, k_imag, v_real, v_imag)]
    W_qkv = np.asarray(W_qkv, np.float32)
    b_qkv = np.asarray(b_qkv, np.float32)
    W_out = np.asarray(W_out, np.float32)
    b_out = np.asarray(b_out, np.float32)

    zb = bool(not b_qkv.any() and not b_out.any())
    preps = [_prep_mask(mask_real), _prep_mask(mask_imag)]
    programs = [build_program(*p[:4], zero_bias=zb) for p in preps]

    # x^T per batch, c-tiled hi/lo: [CT, 128, L] e4m3 + e5m2
    x8_ts, xl_ts = [], []
    for b in range(B):
        xb = np.concatenate([a[b] for a in args], axis=1)          # [L, 6D]
        xt = np.ascontiguousarray(xb.T.astype(np.float32))          # [6D, L]
        xh, xl = _split8(xt)
        x8_ts.append(np.ascontiguousarray(xh.reshape(CT, 128, L)))
        xl_ts.append(np.ascontiguousarray(xl.reshape(CT, 128, L)))

    W6T = W_qkv.T  # [c, f]
    W2T = W_out.T  # [f=2D, g=2D]
    stream_inputs = []
    for s in range(2):
        wq = W6T[:, _Q_OFF[s]:_Q_OFF[s] + D].reshape(CT, 128, 512)
        wk = W6T[:, _K_OFF[s]:_K_OFF[s] + D].reshape(CT, 128, 512)
        wv = W6T[:, _V_OFF[s]:_V_OFF[s] + D].reshape(CT, 128, 512)
        wqk = np.ascontiguousarray(np.stack([wq, wk], axis=2))     # [CT,128,2,512]
        wqkh, wqkl = _split8(wqk)
        wvh, wvl = _split8(np.ascontiguousarray(wv))
        bq = b_qkv[_Q_OFF[s]:_Q_OFF[s] + D].reshape(4, 128).T
        bk = b_qkv[_K_OFF[s]:_K_OFF[s] + D].reshape(4, 128).T
        b_qks = np.ascontiguousarray(
            np.concatenate([bq, bk], axis=1), dtype=np.float32)    # [128, 8]
        wo_r = W2T[s * D:(s + 1) * D, :].reshape(4, 128, 2 * D)
        w_os = np.ascontiguousarray(wo_r.astype(BF))
        woh, wol = _split8(np.ascontiguousarray(wo_r))
        if s == 0:
            b_v_cat = np.concatenate([b_qkv[_V_OFF[0]:_V_OFF[0] + D],
                                      b_qkv[_V_OFF[1]:_V_OFF[1] + D]])
            b_eff = (W_out @ b_v_cat + b_out).astype(np.float32)
            b_ys = np.ascontiguousarray(b_eff.reshape(8, 128).T)
        else:
            b_ys = np.zeros((128, 8), np.float32)
        stream_inputs.append(dict(
            wqkh=np.ascontiguousarray(wqkh), wqkl=np.ascontiguousarray(wqkl),
            wvh=np.ascontiguousarray(wvh), wvl=np.ascontiguousarray(wvl),
            b_qks=b_qks, w_os=w_os, b_ys=b_ys,
            woh=np.ascontiguousarray(woh), wol=np.ascontiguousarray(wol)))

    LAST_RESULTS = []
    LAST_PROGRAMS = programs
    stream_res = []
    for s in range(2):
        si = stream_inputs[s]
        in_maps = []
        for b in range(B):
            in_maps.append({
                "x8_t": x8_ts[b], "xl_t": xl_ts[b],
                "wqkh_t": si["wqkh"], "wqkl_t": si["wqkl"],
                "wvh_t": si["wvh"], "wvl_t": si["wvl"],
                "w_o": si["w_os"], "woh_t": si["woh"], "wol_t": si["wol"],
                "b_qk": si["b_qks"], "b_y": si["b_ys"],
                "b_yr": np.ascontiguousarray(
                    si["b_ys"].T.reshape(1, 2 * D).astype(BF)),
                "mask_t": preps[s][4],
                "ones_a": np.ones((128, 1), BF),
                "ones_b": np.ones((1, 128), BF),
            })
        res = run_bass_kernel_spmd(programs[s], in_maps,
                                   core_ids=[4 * s + b for b in range(B)],
                                   trace=_trace)
        LAST_RESULTS.append(res)
        stream_res.append(res)

    out_real = np.empty((B, L, D), np.float32)
    out_imag = np.empty((B, L, D), np.float32)
    for b in range(B):
        yt = stream_res[0].results[b]["y"] + stream_res[1].results[b]["y"]
        yb = yt.T                                                   # [L, 2D]
        out_real[b] = yb[:, :D]
        out_imag[b] = yb[:, D:]
    return out_real, out_imag


# revision 31
# speedup vs baseline: 1.2654x; 1.0174x over previous
"""Bass/Trainium2 kernel for nn_BasicQuantumAttention (B=4, L=2048, d=512, 8 cores).

Sharding: core (b, s) = batch b, stream s (real/imag); one program per
stream (each stream's own block-sparse keep-set; ~52/60 kept 128x128
tiles vs 79 for the union). Each core:
  - projects x[b] -> qT, kT (layout [d, L]) and v (layout [L, d]), all
    SBUF-resident. The projection runs as fp8 DoubleRow matmuls with
    hi/lo error compensation: every operand A is split into
    A_hi = e4m3(A) and A_lo = e5m2(A - A_hi), and A@B is computed as
    three DoubleRow pass chains (Ah@Bh, Ah@Bl, Al@Bh). DoubleRow
    contracts two 128-K slabs per instruction, so the three passes cost
    0.75x the bf16 cycles while matching bf16 accuracy (the dropped
    Al@Bl term is ~2^-8 relative). q/k are evicted from PSUM as
    (e4m3 hi, e5m2 lo) pairs so the score matmuls use the same scheme;
    v is evicted bf16 for the (bf16) attnV matmuls.
  - block-sparse masked attention with compile-time tile skipping at
    128x128 granularity on this stream's mask; scores are fp8 tri-term
    DoubleRow, exp/mask/attnV as in the bf16 kernel.
  - partial out-projection y^T_part = W_out^T[stream rows].T @ O_norm^T
    (bf16).
Host sums the two partial y^T per batch and untransposes.
"""
import sys

sys.path.insert(0, "/opt/trn_rl_repo")

import numpy as np
import ml_dtypes

import concourse.bass as bass
import concourse.tile as tile
from concourse import bacc, mybir
from concourse.bass_utils import run_bass_kernel_spmd

B, L, D = 4, 2048, 512
C6 = 6 * D            # 3072 input features
CT = C6 // 128        # 24 contraction tiles
QS = L // 512         # 4 query slices of 512 (normalization/out-proj grain)
QB = L // 128         # 16 query blocks of 128 (attention grain)
KT = L // 128         # 16 key tiles of 128
F32 = mybir.dt.float32
BF16 = mybir.dt.bfloat16
F8H = mybir.dt.float8e4
F8L = mybir.dt.float8e5
DR = mybir.MatmulPerfMode.DoubleRow
SCALE = float(D) ** -0.5
BF = ml_dtypes.bfloat16
H8 = ml_dtypes.float8_e4m3
E5 = ml_dtypes.float8_e5m2

# feature offsets inside qkv = [q_r q_i k_r k_i v_r v_i] (each D wide)
_Q_OFF = {0: 0 * D, 1: 1 * D}
_K_OFF = {0: 2 * D, 1: 3 * D}
_V_OFF = {0: 4 * D, 1: 5 * D}

LAST_RESULTS = None   # list of per-stream BassKernelResults
LAST_PROGRAMS = None  # list of per-stream compiled Bacc programs


def build_program(kept, needs_mask, slot_of, n_slots, zero_bias=False):
    """kept: {qb_global: [kt,...]} keep lists at 128x128 granularity for
    THIS stream; needs_mask: set[(qb,kt)]; slot_of: {(qb,kt): slot}."""
    nc = bacc.Bacc(None, target_bir_lowering=False, debug=False)

    x8_t = nc.dram_tensor("x8_t", [CT, 128, L], F8H, kind="ExternalInput")
    xl_t = nc.dram_tensor("xl_t", [CT, 128, L], F8L, kind="ExternalInput")
    wqkh_t = nc.dram_tensor("wqkh_t", [CT, 128, 2, 512], F8H, kind="ExternalInput")
    wqkl_t = nc.dram_tensor("wqkl_t", [CT, 128, 2, 512], F8L, kind="ExternalInput")
    wvh_t = nc.dram_tensor("wvh_t", [CT, 128, 512], F8H, kind="ExternalInput")
    wvl_t = nc.dram_tensor("wvl_t", [CT, 128, 512], F8L, kind="ExternalInput")
    w_o = nc.dram_tensor("w_o", [4, 128, 2 * D], BF16, kind="ExternalInput")
    woh_t = nc.dram_tensor("woh_t", [4, 128, 2 * D], F8H, kind="ExternalInput")
    wol_t = nc.dram_tensor("wol_t", [4, 128, 2 * D], F8L, kind="ExternalInput")
    b_qk = nc.dram_tensor("b_qk", [128, 8], F32, kind="ExternalInput")
    b_y = nc.dram_tensor("b_y", [128, 8], F32, kind="ExternalInput")
    mask_t = nc.dram_tensor("mask_t", [n_slots, 128, 128], F8H, kind="ExternalInput")
    ones_a = nc.dram_tensor("ones_a", [128, 1], BF16, kind="ExternalInput")
    ones_b = nc.dram_tensor("ones_b", [1, 128], BF16, kind="ExternalInput")
    b_yr = nc.dram_tensor("b_yr", [1, 2 * D], BF16, kind="ExternalInput")
    y = nc.dram_tensor("y", [2 * D, L], F32, kind="ExternalOutput")

    with tile.TileContext(nc) as tc, \
         nc.allow_low_precision(reason="fp8 hi/lo compensated matmuls"):
        with tc.tile_pool(name="consts", bufs=1) as consts, \
             tc.tile_pool(name="kqv", bufs=1) as kqv:
            ones_k = consts.tile([128, 1], BF16)
            ones_1 = consts.tile([1, 128], BF16)
            bqk_s = consts.tile([128, 8], F32)
            by_s = consts.tile([128, 8], F32)
            byr_s = consts.tile([1, 2 * D], BF16)
            if zero_bias:
                woh_sb = consts.tile([128, 4, 2 * D], F8H)
                wol_sb = consts.tile([128, 4, 2 * D], F8L)
            else:
                wo_sb = consts.tile([128, 4, 2 * D], BF16)
            # all mask tiles live in SBUF for the whole program (fp8: 0/1 is
            # exact), loaded with ONE transfer during the projection phase —
            # per-job mask DMAs would serialize on HWDGE (625ns each) and
            # add ~1.5us of latency in front of every masked attnV
            mask_sb = consts.tile([128, n_slots, 128], F8H)

            qh_sb = kqv.tile([128, 4, L], F8H)
            ql_sb = kqv.tile([128, 4, L], F8L)
            kh_sb = kqv.tile([128, 4, L], F8H)
            kl_sb = kqv.tile([128, 4, L], F8L)
            v_sb = kqv.tile([128, KT, 512], BF16)

            # ---------------- projection phase ----------------
            with tc.tile_pool(name="wc", bufs=1) as wcp, \
                 tc.tile_pool(name="xin", bufs=2) as xp, \
                 tc.tile_pool(name="ev", bufs=4) as evp, \
                 tc.tile_pool(name="pp", bufs=8, space="PSUM") as pp:
                wqkh_sb = wcp.tile([128, CT, 2, 512], F8H)
                wqkl_sb = wcp.tile([128, CT, 2, 512], F8L)
                wvh_sb = wcp.tile([128, CT, 512], F8H)
                wvl_sb = wcp.tile([128, CT, 512], F8L)

                x_tiles = {}

                def load_x(qs_, pieces=1):
                    xh = xp.tile([128, CT, 512], F8H, name=f"x8{qs_}", tag="x8")
                    xl = xp.tile([128, CT, 512], F8L, name=f"xl{qs_}", tag="xl")
                    sl = slice(qs_ * 512, (qs_ + 1) * 512)
                    step = CT // pieces
                    for c0 in range(0, CT, step):
                        cs = slice(c0, c0 + step)
                        nc.sync.dma_start(
                            out=xh[:, cs, :],
                            in_=x8_t[cs, :, sl].rearrange("ct p n -> p ct n"))
                        nc.sync.dma_start(
                            out=xl[:, cs, :],
                            in_=xl_t[cs, :, sl].rearrange("ct p n -> p ct n"))
                    x_tiles[qs_] = (xh, xl)

                # PE p-state warm-up: burn the 0.65->2.4GHz ramp on dummy
                # matmuls while the first input DMAs are in flight
                warm = consts.tile([128, 128], BF16)
                nc.vector.memset(warm, 0.0)
                wps = pp.tile([128, 512], F32, name="wps", tag="ps")
                for _ in range(8):
                    nc.tensor.matmul(wps[:, 0:128], warm[:, :], warm[:, :],
                                     start=True, stop=True)

                x0h = xp.tile([128, CT, 512], F8H, name="x80", tag="x8")
                x0l = xp.tile([128, CT, 512], F8L, name="xl0", tag="xl")
                # qs=0 runs wave1 = q+k (8 chains, consuming wqk hi/lo + x
                # hi/lo at ~1.16us/ct delivered vs 1.28us/ct consumed) then
                # wave2 = v (wv hi/lo, delivered during wave1's tail). The
                # DMA engines are one serialized resource, so the sync
                # stream is ordered exactly in consumption order; only the
                # first x8 piece + consts ride gpsimd for issue parallelism.
                nc.gpsimd.dma_start(out=x0h[:, 0:2, :],
                                    in_=x8_t[0:2, :, 0:512].rearrange(
                                        "ct p n -> p ct n"))
                nc.sync.dma_start(
                    out=wqkh_sb[:, 0:2, :, :],
                    in_=wqkh_t[0:2].rearrange("ct p f d -> p ct f d"))
                nc.sync.dma_start(
                    out=wqkl_sb[:, 0:2, :, :],
                    in_=wqkl_t[0:2].rearrange("ct p f d -> p ct f d"))
                nc.sync.dma_start(out=x0l[:, 0:2, :],
                                  in_=xl_t[0:2, :, 0:512].rearrange(
                                      "ct p n -> p ct n"))
                # consts trail on gpsimd (nothing needs them until the first
                # evictions ~15us in)
                nc.gpsimd.dma_start(out=ones_k, in_=ones_a[:, :])
                nc.gpsimd.dma_start(out=ones_1, in_=ones_b[:, :])
                nc.gpsimd.dma_start(out=bqk_s, in_=b_qk[:, :])
                nc.gpsimd.dma_start(out=by_s, in_=b_y[:, :])
                nc.gpsimd.dma_start(out=byr_s, in_=b_yr[:, :])
                # pre-warm the exp activation table while PE projects
                scrap = consts.tile([128, 8], BF16)
                nc.scalar.activation(out=scrap, in_=bqk_s,
                                     func=mybir.ActivationFunctionType.Exp)
                ct_groups = [[2, 3, 4, 5]] + \
                    [list(range(c, min(c + 6, CT))) for c in range(6, CT, 6)]
                for grp_ in ct_groups:
                    c0, cn = grp_[0], len(grp_)
                    nc.sync.dma_start(
                        out=wqkh_sb[:, c0:c0 + cn, :, :],
                        in_=wqkh_t[c0:c0 + cn].rearrange("ct p f d -> p ct f d"))
                    nc.sync.dma_start(
                        out=wqkl_sb[:, c0:c0 + cn, :, :],
                        in_=wqkl_t[c0:c0 + cn].rearrange("ct p f d -> p ct f d"))
                    nc.sync.dma_start(
                        out=x0h[:, c0:c0 + cn, :],
                        in_=x8_t[c0:c0 + cn, :, 0:512].rearrange(
                            "ct p n -> p ct n"))
                    nc.sync.dma_start(
                        out=x0l[:, c0:c0 + cn, :],
                        in_=xl_t[c0:c0 + cn, :, 0:512].rearrange(
                            "ct p n -> p ct n"))
                # wv hi/lo stream for wave2, in consumption order
                for c0 in range(0, CT, 6):
                    cn = min(6, CT - c0)
                    nc.sync.dma_start(
                        out=wvh_sb[:, c0:c0 + cn, :],
                        in_=wvh_t[c0:c0 + cn].rearrange("ct p d -> p ct d"))
                    nc.sync.dma_start(
                        out=wvl_sb[:, c0:c0 + cn, :],
                        in_=wvl_t[c0:c0 + cn].rearrange("ct p d -> p ct d"))
                # one-shot mask preload (needed from the first attention job)
                nc.sync.dma_start(out=mask_sb,
                                  in_=mask_t.rearrange("s p n -> p s n"))
                x_tiles[0] = (x0h, x0l)
                # x[1] is consumed right as its delivery completes: split it
                # into pieces so the sem fires progressively
                load_x(1, pieces=3)

                def evict(kind, ft, ps, qs_):
                    if kind == "v":
                        # v bias is folded into b_y on the host (as in the
                        # bf16 kernel), so v eviction is always a plain copy
                        nc.scalar.copy(out=v_sb[:, qs_ * 4 + ft, :], in_=ps)
                        return
                    hi, lo = (qh_sb, ql_sb) if kind == "q" else (kh_sb, kl_sb)
                    bi = ft if kind == "q" else 4 + ft
                    sl = slice(qs_ * 512, (qs_ + 1) * 512)
                    if zero_bias:
                        nc.scalar.copy(out=hi[:, ft, sl], in_=ps)
                        nc.vector.tensor_sub(lo[:, ft, sl], ps, hi[:, ft, sl])
                    else:
                        tmp = evp.tile([128, 512], BF16, name="evt", tag="evt")
                        nc.scalar.activation(
                            out=tmp, in_=ps,
                            func=mybir.ActivationFunctionType.Identity,
                            bias=bqk_s[:, bi:bi + 1])
                        nc.scalar.copy(out=hi[:, ft, sl], in_=tmp)
                        nc.vector.tensor_sub(lo[:, ft, sl], tmp, hi[:, ft, sl])

                def mm_steps(kind, ft, c, xt):
                    """The 3 DoubleRow (lhsT, rhs) pairs for ct-pair c."""
                    xh, xl = xt
                    cp = slice(c, c + 2)
                    fsl = slice(ft * 128, (ft + 1) * 128)
                    if kind == "v":
                        return [(xh[:, cp, fsl], wvh_sb[:, cp, :]),
                                (xh[:, cp, fsl], wvl_sb[:, cp, :]),
                                (xl[:, cp, fsl], wvh_sb[:, cp, :])]
                    fc = 0 if kind == "q" else 1
                    return [(wqkh_sb[:, cp, fc, fsl], xh[:, cp, :]),
                            (wqkl_sb[:, cp, fc, fsl], xh[:, cp, :]),
                            (wqkh_sb[:, cp, fc, fsl], xl[:, cp, :])]

                NP = 3 * (CT // 2)  # matmuls per chain

                # qs=0: ct-pair-major waves so PE consumption tracks DMA
                # delivery; wave1 = q+k (needs wqk+x, 8 PSUM banks),
                # wave2 = v (wv lands during wave1)
                x0 = x_tiles.pop(0)
                waves = [[("q", ft) for ft in range(4)] +
                         [("k", ft) for ft in range(4)],
                         [("v", nt) for nt in range(4)]]
                for wave in waves:
                    pss = {u: pp.tile([128, 512], F32, name=f"ps{u[0]}{u[1]}",
                                      tag="ps") for u in wave}
                    cnt = {u: 0 for u in wave}
                    for c in range(0, CT, 2):
                        for u in wave:
                            for lhsT, rhs in mm_steps(u[0], u[1], c, x0):
                                nc.tensor.matmul(
                                    pss[u][:, :], lhsT, rhs,
                                    start=(cnt[u] == 0),
                                    stop=(cnt[u] == NP - 1), perf_mode=DR)
                                cnt[u] += 1
                    for u in wave:
                        evict(u[0], u[1], pss[u], 0)

                for qs in range(1, QS):
                    if qs + 1 < QS:
                        load_x(qs + 1)
                    x_qs = x_tiles.pop(qs)
                    for kind in ("q", "k", "v"):
                        for ft in range(4):
                            ps = pp.tile([128, 512], F32, name="ps", tag="ps")
                            i = 0
                            for c in range(0, CT, 2):
                                for lhsT, rhs in mm_steps(kind, ft, c, x_qs):
                                    nc.tensor.matmul(
                                        ps[:, :], lhsT, rhs,
                                        start=(i == 0), stop=(i == NP - 1),
                                        perf_mode=DR)
                                    i += 1
                            evict(kind, ft, ps, qs)

            # ---------------- attention + out-projection ----------------
            jobs = []   # (qs, qb, [groups of up to 4 kt])
            for qs in range(QS):
                # biggest jobs first within each qs: the overall last job is
                # then small, shrinking the end-of-program exp/attnV drain
                order = sorted(range(4),
                               key=lambda qb: -len(kept[qs * 4 + qb]))
                for qb in order:
                    klist = kept[qs * 4 + qb]
                    jobs.append((qs, qb,
                                 [klist[i:i + 4]
                                  for i in range(0, len(klist), 4)]))

            with tc.tile_pool(name="sy", bufs=3, space="PSUM") as syp, \
                 tc.tile_pool(name="op", bufs=2, space="PSUM") as opp, \
                 tc.tile_pool(name="dn", bufs=1, space="PSUM") as dnp, \
                 tc.tile_pool(name="yp", bufs=2, space="PSUM") as ypp, \
                 tc.tile_pool(name="pt", bufs=8) as ptp, \
                 tc.tile_pool(name="ot", bufs=3) as otp, \
                 tc.tile_pool(name="ot8", bufs=2) as ot8p, \
                 tc.tile_pool(name="sm", bufs=2) as smp, \
                 tc.tile_pool(name="yo", bufs=4) as yop:
                if zero_bias:
                    nc.scalar.dma_start(
                        out=woh_sb, in_=woh_t.rearrange("ft p g -> p ft g"))
                    nc.scalar.dma_start(
                        out=wol_sb, in_=wol_t.rearrange("ft p g -> p ft g"))
                else:
                    nc.scalar.dma_start(
                        out=wo_sb, in_=w_o.rearrange("ft p g -> p ft g"))

                state = {}   # per-qs tiles: ot, dps

                def emit_scores_grp(qs, qb, grp):
                    g = qs * 4 + qb
                    w = len(grp)
                    gsl = slice(g * 128, (g + 1) * 128)
                    sps = syp.tile([128, 4, 128], F32, name="sps", tag="sps")
                    for j, kt in enumerate(grp):
                        ksl = slice(kt * 128, (kt + 1) * 128)
                        steps = []
                        for dt in (0, 2):
                            steps.append((kh_sb[:, dt:dt + 2, ksl],
                                          qh_sb[:, dt:dt + 2, gsl]))
                        for dt in (0, 2):
                            steps.append((kh_sb[:, dt:dt + 2, ksl],
                                          ql_sb[:, dt:dt + 2, gsl]))
                        for dt in (0, 2):
                            steps.append((kl_sb[:, dt:dt + 2, ksl],
                                          qh_sb[:, dt:dt + 2, gsl]))
                        for i, (lhsT, rhs) in enumerate(steps):
                            nc.tensor.matmul(
                                sps[:, j, :], lhsT, rhs,
                                start=(i == 0), stop=(i == len(steps) - 1),
                                perf_mode=DR)
                    pT = ptp.tile([128, 4, 128], BF16, name="pT", tag="pT")
                    nc.scalar.activation(
                        out=pT[:, :w, :], in_=sps[:, :w, :],
                        func=mybir.ActivationFunctionType.Exp, scale=SCALE)
                    masked = [j for j, kt in enumerate(grp)
                              if (g, kt) in needs_mask]
                    if masked:
                        slots = [slot_of[(g, grp[j])] for j in masked]
                        contig = (len(masked) == masked[-1] - masked[0] + 1
                                  and slots == list(range(slots[0],
                                                          slots[0] + len(slots))))
                        if contig:
                            j0, sw = masked[0], len(masked)
                            nc.vector.tensor_mul(
                                pT[:, j0:j0 + sw, :], pT[:, j0:j0 + sw, :],
                                mask_sb[:, slots[0]:slots[0] + sw, :])
                        else:
                            for i, j in enumerate(masked):
                                nc.vector.tensor_mul(
                                    pT[:, j, :], pT[:, j, :],
                                    mask_sb[:, slots[i], :])
                    return pT

                def emit_final_recip(qs, qb):
                    """recip AND the rb broadcast, emitted before the attnV
                    chains so the cross-engine ladder (DVE recip -> PE bps ->
                    ACT rb) completes while PE runs attnV."""
                    dps = state[("dps", qs)]
                    c0, c1 = qb * 128, (qb + 1) * 128
                    recf = smp.tile([1, 128], F32, tag="recf", name="recf")
                    nc.vector.reciprocal(recf, dps[:, c0:c1])
                    recs = smp.tile([1, 128], BF16, tag="recs", name="recs")
                    nc.vector.tensor_copy(out=recs, in_=recf)
                    bps = syp.tile([128, 128], F32, tag="sps", name="bps")
                    nc.tensor.matmul(bps[:, :], ones_1[:, :], recs[:, :],
                                     start=True, stop=True)
                    rb = smp.tile([128, 128], BF16, tag="rbq", name="rbq")
                    nc.scalar.copy(out=rb, in_=bps)
                    return rb

                def emit_final_qb_head(qs, qb, rb, last):
                    ot = state[("ot", qs)]
                    c0, c1 = qb * 128, (qb + 1) * 128
                    for ft in range(4):
                        nc.vector.tensor_mul(ot[:, ft, c0:c1],
                                             ot[:, ft, c0:c1], rb)
                    if zero_bias:
                        oth = ot8p.tile([128, 4, 128], F8H, name="othq",
                                        tag="othq")
                        otl = ot8p.tile([128, 4, 128], F8L, name="otlq",
                                        tag="otlq")
                        nc.scalar.copy(out=oth, in_=ot[:, :, c0:c1])
                        nc.vector.tensor_sub(otl, ot[:, :, c0:c1], oth)
                    for gh in range(2):
                        ypsg = ypp.tile([128, 4, 128], F32, tag="yp", name="ypsg")
                        for gi in range(4):
                            gt = gh * 4 + gi
                            gsl = slice(gt * 128, (gt + 1) * 128)
                            if zero_bias:
                                steps = []
                                for ft in (0, 2):
                                    steps.append((woh_sb[:, ft:ft + 2, gsl],
                                                  oth[:, ft:ft + 2, :]))
                                for ft in (0, 2):
                                    steps.append((woh_sb[:, ft:ft + 2, gsl],
                                                  otl[:, ft:ft + 2, :]))
                                for ft in (0, 2):
                                    steps.append((wol_sb[:, ft:ft + 2, gsl],
                                                  oth[:, ft:ft + 2, :]))
                                for i, (lh, rh) in enumerate(steps):
                                    nc.tensor.matmul(
                                        ypsg[:, gi, :], lh, rh,
                                        start=(i == 0), stop=(i == 5),
                                        perf_mode=DR)
                            else:
                                for ft in range(4):
                                    nc.tensor.matmul(
                                        ypsg[:, gi, :],
                                        wo_sb[:, ft, gsl],
                                        ot[:, ft, c0:c1],
                                        start=(ft == 0), stop=False)
                                nc.tensor.matmul(
                                    ypsg[:, gi, :],
                                    byr_s[:, gsl],
                                    ones_1[:, :], start=False, stop=True)
                        y_sbq = yop.tile([128, 4, 128], F32,
                                         name="ysq", tag="ysq")
                        if last and gh == 1:
                            nc.vector.tensor_copy(out=y_sbq, in_=ypsg)
                        else:
                            nc.scalar.copy(out=y_sbq, in_=ypsg)
                        yq = (nc.sync, nc.scalar)[gh] if last else \
                            (nc.sync, nc.gpsimd)[gh]
                        yq.dma_start(
                            out=y[gh * 512:(gh + 1) * 512,
                                  qs * 512 + c0:qs * 512 + c1].rearrange(
                                      "(a p) n -> p a n", p=128),
                            in_=y_sbq)

                def emit_attnv_job(job, pts, last):
                    qs, qb, groups = job
                    dps = state[("dps", qs)]
                    n = sum(len(g) for g in groups)
                    i = 0
                    for gi, grp in enumerate(groups):
                        for j, kt in enumerate(grp):
                            nc.tensor.matmul(
                                dps[:, qb * 128:(qb + 1) * 128],
                                ones_k[:, :], pts[gi][:, j, :],
                                start=(i == 0), stop=(i == n - 1))
                            i += 1
                    rb = None
                    if qs == QS - 1:
                        rb = emit_final_recip(qs, qb)
                    ops = opp.tile([128, 4, 128], F32, name="ops", tag="ops")
                    for dvt in range(4):
                        i = 0
                        for gi, grp in enumerate(groups):
                            for j, kt in enumerate(grp):
                                nc.tensor.matmul(
                                    ops[:, dvt, :],
                                    v_sb[:, kt, dvt * 128:(dvt + 1) * 128],
                                    pts[gi][:, j, :],
                                    start=(i == 0), stop=(i == n - 1))
                                i += 1
                    nc.vector.tensor_copy(
                        out=state[("ot", qs)][:, :, qb * 128:(qb + 1) * 128],
                        in_=ops)
                    if qs == QS - 1:
                        emit_final_qb_head(qs, qb, rb, last)

                def head_a(qs):
                    """recip + rb broadcast (cross-engine ladder, overlaps
                    the next job's PE work)."""
                    dps = state.pop(("dps", qs))
                    recip = smp.tile([1, 512], F32, tag="recf", name="recip")
                    nc.vector.reciprocal(recip, dps)
                    recs = smp.tile([1, 512], BF16, tag="recs", name="recs")
                    nc.vector.tensor_copy(out=recs, in_=recip)
                    bps = ypp.tile([128, 512], F32, tag="yp", name="bps")
                    nc.tensor.matmul(bps[:, :], ones_1[:, :], recs[:, :],
                                     start=True, stop=True)
                    rb = smp.tile([128, 512], BF16, tag="rb", name="rb")
                    nc.scalar.copy(out=rb, in_=bps)
                    state[("rb", qs)] = rb

                def head_b1(qs):
                    """normalize ot + hi/lo split (ACT/DVE work, one retire
                    ahead of the PE-dense yps chains)."""
                    if not zero_bias:
                        return
                    ot = state[("ot", qs)]
                    rb = state[("rb", qs)]
                    for ft in range(4):
                        nc.vector.tensor_mul(ot[:, ft, :], ot[:, ft, :], rb)
                    oth = ot8p.tile([128, 4, 512], F8H, name="oth", tag="oth")
                    otl = ot8p.tile([128, 4, 512], F8L, name="otl", tag="otl")
                    nc.scalar.copy(out=oth, in_=ot)
                    nc.vector.tensor_sub(otl, ot, oth)
                    state[("oth", qs)] = oth
                    state[("otl", qs)] = otl

                def head_b2(qs):
                    ot = state.pop(("ot", qs))
                    rb = state.pop(("rb", qs))
                    if zero_bias:
                        oth = state.pop(("oth", qs))
                        otl = state.pop(("otl", qs))
                    yqs = ([nc.sync, nc.gpsimd, nc.scalar, nc.sync]
                           if qs == QS - 1 else
                           [nc.gpsimd, nc.scalar, nc.gpsimd, nc.scalar])
                    for gp in range(4):
                        y_sb = yop.tile([128, 2, 512], F32,
                                        name="y_sb", tag="y_sb")
                        for gi in range(2):
                            gt = gp * 2 + gi
                            gsl = slice(gt * 128, (gt + 1) * 128)
                            yps = ypp.tile([128, 512], F32, tag="yp", name="yps")
                            if zero_bias:
                                steps = []
                                for ft in (0, 2):
                                    steps.append((woh_sb[:, ft:ft + 2, gsl],
                                                  oth[:, ft:ft + 2, :]))
                                for ft in (0, 2):
                                    steps.append((woh_sb[:, ft:ft + 2, gsl],
                                                  otl[:, ft:ft + 2, :]))
                                for ft in (0, 2):
                                    steps.append((wol_sb[:, ft:ft + 2, gsl],
                                                  oth[:, ft:ft + 2, :]))
                                for i, (lh, rh) in enumerate(steps):
                                    nc.tensor.matmul(
                                        yps[:, :], lh, rh,
                                        start=(i == 0), stop=(i == 5),
                                        perf_mode=DR)
                                nc.scalar.copy(out=y_sb[:, gi, :], in_=yps)
                            else:
                                for ft in range(4):
                                    nc.tensor.matmul(
                                        yps[:, :],
                                        wo_sb[:, ft, gsl],
                                        ot[:, ft, :], start=(ft == 0),
                                        stop=(ft == 3))
                                ym = yop.tile([128, 512], F32, name="ym",
                                              tag="ym")
                                nc.vector.tensor_mul(ym, yps, rb)
                                nc.scalar.activation(
                                    out=y_sb[:, gi, :], in_=ym,
                                    func=mybir.ActivationFunctionType.Identity,
                                    bias=by_s[:, gt:gt + 1])
                        yqs[gp].dma_start(
                            out=y[gp * 256:(gp + 1) * 256,
                                  qs * 512:(qs + 1) * 512].rearrange(
                                      "(a p) n -> p a n", p=128),
                            in_=y_sb)

                heads = []
                HEAD_STAGES = (head_a, head_b1, head_b2)

                def step_heads():
                    for h in list(heads):
                        HEAD_STAGES[h[1]](h[0])
                        h[1] += 1
                        if h[1] == len(HEAD_STAGES):
                            heads.remove(h)

                done_cnt = {}

                def retire(job, pts, last=False):
                    emit_attnv_job(job, pts, last)
                    qs = job[0]
                    done_cnt[qs] = done_cnt.get(qs, 0) + 1
                    if done_cnt[qs] == 4:
                        if qs == QS - 1:
                            state.pop(("ot", qs), None)
                            state.pop(("dps", qs), None)
                        else:
                            heads.append([qs, 0])
                    step_heads()

                prev = None
                for job in jobs:
                    qs, qb, groups = job
                    if ("ot", qs) not in state:
                        state[("ot", qs)] = otp.tile(
                            [128, 4, 512], BF16, name="ot", tag="ot")
                        state[("dps", qs)] = dnp.tile(
                            [1, 512], F32, name="dps", tag="dps")
                    pts = [emit_scores_grp(qs, qb, grp) for grp in groups]
                    if prev is not None:
                        retire(*prev)
                    prev = (job, pts)
                retire(*prev, last=True)
                while heads:
                    step_heads()

    nc.compile()
    return nc


def _prep_mask(mask):
    """Compile-time 128x128 tile analysis for ONE stream's mask."""
    mt = np.ascontiguousarray(np.asarray(mask).T)
    kept = {}
    needs_mask = set()
    slot_of = {}
    slots = []  # (qb, kt)
    for g in range(QB):
        klist = []
        for kt in range(KT):
            sub = mt[kt * 128:(kt + 1) * 128, g * 128:(g + 1) * 128]
            if not sub.any():
                continue
            klist.append(kt)
            if not sub.all():
                needs_mask.add((g, kt))
                slot_of[(g, kt)] = len(slots)
                slots.append((g, kt))
        kept[g] = klist
    n_slots = max(1, len(slots))
    md = np.zeros((n_slots, 128, 128), H8)
    for i, (g, kt) in enumerate(slots):
        md[i] = mt[kt * 128:(kt + 1) * 128,
                   g * 128:(g + 1) * 128].astype(H8)
    return kept, needs_mask, slot_of, n_slots, md


def _split8(a):
    h = a.astype(H8)
    l = (a - h.astype(np.float32)).astype(E5)
    return h, l


def kernel(q_real, q_imag, k_real, k_imag, v_real, v_imag,
           W_qkv, b_qkv, W_out, b_out, mask_real, mask_imag, _trace=False):
    global LAST_RESULTS, LAST_PROGRAMS
    args = [np.asarray(a) for a in (q_real, q_imag, k_real, k_imag, v_real, v_imag)]
    W_qkv = np.asarray(W_qkv, np.float32)
    b_qkv = np.asarray(b_qkv, np.float32)
    W_out = np.asarray(W_out, np.float32)
    b_out = np.asarray(b_out, np.float32)

    zb = bool(not b_qkv.any() and not b_out.any())
    preps = [_prep_mask(mask_real), _prep_mask(mask_imag)]
    programs = [build_program(*p[:4], zero_bias=zb) for p in preps]

    # x^T per batch, c-tiled hi/lo: [CT, 128, L] e4m3 + e5m2
    x8_ts, xl_ts = [], []
    for b in range(B):
        xb = np.concatenate([a[b] for a in args], axis=1)          # [L, 6D]
        xt = np.ascontiguousarray(xb.T.astype(np.float32))          # [6D, L]
        xh, xl = _split8(xt)
        x8_ts.append(np.ascontiguousarray(xh.reshape(CT, 128, L)))
        xl_ts.append(np.ascontiguousarray(xl.reshape(CT, 128, L)))

    W6T = W_qkv.T  # [c, f]
    W2T = W_out.T  # [f=2D, g=2D]
    stream_inputs = []
    for s in range(2):
        wq = W6T[:, _Q_OFF[s]:_Q_OFF[s] + D].reshape(CT, 128, 512)
        wk = W6T[:, _K_OFF[s]:_K_OFF[s] + D].reshape(CT, 128, 512)
        wv = W6T[:, _V_OFF[s]:_V_OFF[s] + D].reshape(CT, 128, 512)
        wqk = np.ascontiguousarray(np.stack([wq, wk], axis=2))     # [CT,128,2,512]
        wqkh, wqkl = _split8(wqk)
        wvh, wvl = _split8(np.ascontiguousarray(wv))
        bq = b_qkv[_Q_OFF[s]:_Q_OFF[s] + D].reshape(4, 128).T
        bk = b_qkv[_K_OFF[s]:_K_OFF[s] + D].reshape(4, 128).T
        b_qks = np.ascontiguousarray(
            np.concatenate([bq, bk], axis=1), dtype=np.float32)    # [128, 8]
        wo_r = W2T[s * D:(s + 1) * D, :].reshape(4, 128, 2 * D)
        w_os = np.ascontiguousarray(wo_r.astype(BF))
        woh, wol = _split8(np.ascontiguousarray(wo_r))
        if s == 0:
            b_v_cat = np.concatenate([b_qkv[_V_OFF[0]:_V_OFF[0] + D],
                                      b_qkv[_V_OFF[1]:_V_OFF[1] + D]])
            b_eff = (W_out @ b_v_cat + b_out).astype(np.float32)
            b_ys = np.ascontiguousarray(b_eff.reshape(8, 128).T)
        else:
            b_ys = np.zeros((128, 8), np.float32)
        stream_inputs.append(dict(
            wqkh=np.ascontiguousarray(wqkh), wqkl=np.ascontiguousarray(wqkl),
            wvh=np.ascontiguousarray(wvh), wvl=np.ascontiguousarray(wvl),
            b_qks=b_qks, w_os=w_os, b_ys=b_ys,
            woh=np.ascontiguousarray(woh), wol=np.ascontiguousarray(wol)))

    LAST_RESULTS = []
    LAST_PROGRAMS = programs
    stream_res = []
    for s in range(2):
        si = stream_inputs[s]
        in_maps = []
        for b in range(B):
            in_maps.append({
                "x8_t": x8_ts[b], "xl_t": xl_ts[b],
                "wqkh_t": si["wqkh"], "wqkl_t": si["wqkl"],
                "wvh_t": si["wvh"], "wvl_t": si["wvl"],
                "w_o": si["w_os"], "woh_t": si["woh"], "wol_t": si["wol"],
                "b_qk": si["b_qks"], "b_y": si["b_ys"],
                "b_yr": np.ascontiguousarray(
                    si["b_ys"].T.reshape(1, 2 * D).astype(BF)),
                "mask_t": preps[s][4],
                "ones_a": np.ones((128, 1), BF),
                "ones_b": np.ones((1, 128), BF),
            })
        res = run_bass_kernel_spmd(programs[s], in_maps,
                                   core_ids=[4 * s + b for b in range(B)],
                                   trace=_trace)
        LAST_RESULTS.append(res)
        stream_res.append(res)

    out_real = np.empty((B, L, D), np.float32)
    out_imag = np.empty((B, L, D), np.float32)
    for b in range(B):
        yt = stream_res[0].results[b]["y"] + stream_res[1].results[b]["y"]
        yb = yt.T                                                   # [L, 2D]
        out_real[b] = yb[:, :D]
        out_imag[b] = yb[:, D:]
    return out_real, out_imag
